# revision 1
# baseline (speedup 1.0000x reference)
"""Trainium2 Bass kernel for a GNN message-passing layer.

Reference semantics (per edge e = (src j, dst i)):
    m_in  = [x_j, pos_j - pos_i]                 # [E, 6]
    h     = celu(m_in @ f_w1 + f_b1)             # [E, 64]
    msg   = relu(h @ f_w2 + f_b2)                # [E, 64]
    aggr  = segment_max(msg, dst, N); empty -> 0 # [N, 64]
    u     = celu([aggr, x] @ g_w1 + g_b1)
    out   = celu(u @ g_w2 + g_b2)                # [N, 64]

Sharding: nodes are split into 8 contiguous ranges (6250 per core); each core
receives exactly the edges whose dst lands in its range, so the segment-max is
purely local (no collective).  The host does index-only work: it sorts each
core's nodes by in-degree, lays edges out in "rounds" (round r = the r-th edge
of every node that has one), pairs rounds two-high into 128-partition tiles,
and pads with duplicate edges (max is idempotent).  The device then does every
FLOP: the per-edge MLP as three accumulated matmul streams (celu decomposed as
celu(z) = z + relu(-z) + exp(-relu(-z)) - 1, with the linear z term re-streamed
through the combined weight W1@W2 and all constants folded into one bias), a
running tensor_max over the round tiles, and the node MLP with the same trick.
"""

import math
import os
import sys

import numpy as np

N = 50000
E = 1600000
CORES = 8
NCN = N // CORES            # nodes per core
TILE = 512                  # fp32 matmul moving free dim / one PSUM bank
GRP = 1024                  # uniform processing-group width (columns)
NCW = ((NCN + GRP - 1) // GRP) * GRP      # aggr width per core (7168)
SUP = 4096                  # feats DMA staging superblock (columns)
F32 = np.float32


# --------------------------------------------------------------------------
# host-side layout (index work only)
# --------------------------------------------------------------------------

def _core_layouts(edge_index):
    """Per-core node ordering + degree-sorted CSR of local edges."""
    dst = np.asarray(edge_index[1])
    cores = []
    for c in range(CORES):
        lo, hi = c * NCN, (c + 1) * NCN
        eids = np.nonzero((dst >= lo) & (dst < hi))[0]
        ldst = (dst[eids] - lo).astype(np.int64)
        deg = np.bincount(ldst, minlength=NCN)
        order = np.argsort(-deg, kind="stable")         # node ranks
        rank = np.empty(NCN, np.int64)
        rank[order] = np.arange(NCN)
        perm = np.argsort(rank[ldst], kind="stable")
        es = eids[perm]                                  # edges sorted by rank
        deg_s = deg[order]
        row_start = np.zeros(NCN + 1, np.int64)
        np.cumsum(deg_s, out=row_start[1:])
        cores.append(dict(es=es, deg_s=deg_s, row_start=row_start,
                          order=order, empty=order[deg_s == 0] + lo))
    return cores


def _tile_plan(cores):
    """Shared (SPMD-uniform) tile plan.

    All groups are uniform GRP (=1024) columns: matmul instructions carry at
    most ONE hardware sync wait, so every group needs "twin" matmuls with a
    free wait slot for redistributed dependencies (see _build_nc).

    Returns (tiles, groups, S):
      tiles  : list of (pair_round t, k) -- k-th 512-tile of pair-round t
      groups : list of (slot_col0, aggr_col0, fd, is_first_round)
      S      : total slot columns (= 512 * len(tiles))
    """
    rmax = max(int(c["deg_s"][0]) for c in cores)
    n_pairs = (rmax + 1) // 2
    tiles = []
    for t in range(n_pairs):
        w = max(int(np.searchsorted(-c["deg_s"], -(2 * t), side="left"))
                for c in cores)      # max over cores of #nodes with deg > 2t
        k_t = 2 * max(1, (w + GRP - 1) // GRP)       # 512-tiles, even count
        for k in range(k_t):
            tiles.append((t, k))
    S = TILE * len(tiles)

    groups = []
    for i in range(0, len(tiles), 2):
        t, k = tiles[i]
        groups.append((i * TILE, k * TILE, GRP, t == 0))
    return tiles, groups, S


def _pack_core(core, tiles, S, x, pos, src, dst):
    """Build one core's slot->edge assignment and gather features."""
    es, deg_s, row_start = core["es"], core["deg_s"], core["row_start"]
    ncols = len(tiles) * TILE
    nvec = np.tile(np.arange(TILE, dtype=np.int64), len(tiles))  # col in tile
    kvec = np.repeat([k for (_, k) in tiles], TILE)
    tvec = np.repeat([t for (t, _) in tiles], TILE)
    node = kvec * TILE + nvec                    # node rank targeted by column

    safe_node = np.minimum(node, NCN - 1)
    ecap = len(es) - 1
    first_edge = es[np.minimum(row_start[safe_node], ecap)]  # dup fallback
    # nodes with deg 0 or node>=NCN: fall back to edge es[0] (results ignored
    # or fixed up on host)
    bad = (node >= NCN) | (deg_s[safe_node] == 0)
    first_edge = np.where(bad, es[0], first_edge)

    def round_edges(r):
        has = (~bad) & (deg_s[safe_node] > r)
        idx = np.minimum(row_start[safe_node] + np.where(has, r, 0), ecap)
        return np.where(has, es[idx], first_edge)

    a_e = round_edges(2 * tvec)        # vectorized: r differs per column
    b_e = round_edges(2 * tvec + 1)

    feats = np.empty((18, S), dtype=F32)
    for half, eids in ((0, a_e), (9, b_e)):
        s, d = src[eids], dst[eids]
        feats[half + 0:half + 3, :ncols] = x[s].T
        feats[half + 3:half + 6, :ncols] = pos[s].T
        feats[half + 6:half + 9, :ncols] = pos[d].T
    if ncols < S:
        feats[:, ncols:] = 0.0

    xnode = np.zeros((3, NCW), dtype=F32)
    xnode[:, :NCN] = x[core["order"] + 0].T      # caller adds core offset
    return feats, xnode


# column layouts of the packed weight tensors: matmul operands go to a bf16
# pack (PE runs fp32 as ~4 internal passes; bf16 is ~4x faster with f32 PSUM
# accumulation), biases stay f32
WSLOTS = dict(w2=(128, 0, 128), w1n=(18, 128, 128), w12=(18, 256, 128),
              g1n=(67, 384, 64), g12=(67, 448, 64), g2=(64, 512, 64))
WCOL = 576
BSLOTS = dict(nbias1=(128, 0, 1), cbias=(64, 1, 1), nbias_g1=(64, 2, 1),
              nbias_gf=(64, 3, 1), pbias_gf=(64, 4, 1))
BCOL = 8


def _weights(f_w1, f_b1, f_w2, f_b2, g_w1, g_b1, g_w2, g_b2):
    w9 = np.concatenate([f_w1[0:3], f_w1[3:6], -f_w1[3:6]], axis=0)  # [9,64]
    blk = lambda m: np.block([[m, np.zeros_like(m)], [np.zeros_like(m), m]])
    w12 = (w9 @ f_w2).astype(F32)
    cbias = (f_b1 @ f_w2 - f_w2.sum(axis=0) + f_b2).astype(F32)       # [64]
    gbias = (g_b1 @ g_w2 - g_w2.sum(axis=0) + g_b2).astype(F32)       # [64]
    w = dict(
        w1n=blk(-w9).astype(F32),            # [18,128]
        w12=blk(w12),                        # [18,128]
        w2=blk(f_w2).astype(F32),            # [128,128]
        nbias1=np.tile(-f_b1, 2).astype(F32).reshape(128, 1),
        cbias=cbias.reshape(64, 1),
        g1n=(-g_w1).astype(F32),             # [67,64]
        g12=(g_w1 @ g_w2).astype(F32),       # [67,64]
        g2=g_w2.astype(F32),                 # [64,64]
        nbias_g1=(-g_b1).astype(F32).reshape(64, 1),
        nbias_gf=(-gbias).reshape(64, 1),
        pbias_gf=gbias.reshape(64, 1),
    )
    import ml_dtypes
    wpack = np.zeros((128, WCOL), dtype=ml_dtypes.bfloat16)
    for name, (p, c0, cn) in WSLOTS.items():
        wpack[:p, c0:c0 + cn] = w[name]
    bpack = np.zeros((128, BCOL), dtype=F32)
    for name, (p, c0, cn) in BSLOTS.items():
        bpack[:p, c0:c0 + cn] = w[name]
    w["wpack"] = wpack
    w["bpack"] = bpack
    return w


# --------------------------------------------------------------------------
# numpy model of the device program (for validation)
# --------------------------------------------------------------------------

def _numpy_device(feats, xnode, w, groups):
    aggr = np.empty((128, NCW), dtype=F32)
    for (c0, a0, fd, first) in groups:
        f = feats[:, c0:c0 + fd]
        zb = w["w1n"].T.astype(F32) @ f
        msg = w["w12"].T @ f
        r = np.maximum(zb + w["nbias1"], 0).astype(F32)
        e = np.exp(-r).astype(F32)
        msg = msg + w["w2"].T @ r + w["w2"].T @ e
        if first:
            aggr[:, a0:a0 + fd] = msg
        else:
            aggr[:, a0:a0 + fd] = np.maximum(aggr[:, a0:a0 + fd], msg)
    a64 = np.maximum(aggr[0:64], aggr[64:128])
    u_in = np.empty((67, NCW), dtype=F32)
    u_in[0:64] = np.maximum(a64 + w["cbias"], 0)
    u_in[64:67] = xnode
    zg = w["g1n"].T @ u_in
    rg = np.maximum(zg + w["nbias_g1"], 0).astype(F32)
    eg = np.exp(-rg).astype(F32)
    o2 = w["g12"].T @ u_in + w["g2"].T @ rg + w["g2"].T @ eg
    rf = np.maximum(-o2 + w["nbias_gf"], 0).astype(F32)
    ef = np.exp(-rf).astype(F32)
    vf = np.maximum(o2 + w["pbias_gf"], 0).astype(F32)
    return (vf - 1.0 + ef).astype(F32)        # [64, NCW]


# --------------------------------------------------------------------------
# bass program
# --------------------------------------------------------------------------

def _import_concourse():
    try:
        import concourse.bass  # noqa: F401
    except ImportError:
        sys.path.insert(0, "/opt/trn_rl_repo")


def _install_ntff_shim():
    """Provide antenv.axon_hooks (missing in this image) so that
    run_bass_kernel_spmd(trace=True) can capture NTFF profiles through
    libaxon's C ABI (same mechanism as trn_boot's degraded hook)."""
    import contextlib
    import ctypes
    import types

    if "antenv.axon_hooks" in sys.modules:
        return
    so_path = "/opt/axon/libaxon_pjrt.so"
    if not os.path.exists(so_path):
        return
    lib = ctypes.CDLL(so_path)
    if not hasattr(lib, "axon_start_nrt_profile"):
        return
    lib.axon_start_nrt_profile.argtypes = [ctypes.POINTER(ctypes.c_int64),
                                           ctypes.c_size_t]
    lib.axon_start_nrt_profile.restype = ctypes.c_int64
    lib.axon_stop_nrt_profile.argtypes = [ctypes.c_char_p]
    lib.axon_stop_nrt_profile.restype = ctypes.c_int64

    @contextlib.contextmanager
    def _hook(output_dir, device_ids):
        import jax
        jax.devices()
        if device_ids:
            ids = (ctypes.c_int64 * len(device_ids))(*device_ids)
            rc = lib.axon_start_nrt_profile(ids, len(device_ids))
        else:
            rc = lib.axon_start_nrt_profile(None, 0)
        if rc != 0:
            raise RuntimeError(f"axon_start_nrt_profile rc={rc}")
        try:
            yield
        finally:
            n = lib.axon_stop_nrt_profile(str(output_dir).encode())
            print(f"ntff profile: {n} file(s) -> {output_dir}",
                  file=sys.stderr)

    mod = types.ModuleType("antenv.axon_hooks")
    mod.get_axon_ntff_profile_hook = lambda: _hook
    mod.set_axon_ntff_profile_hook = lambda h: None
    sys.modules["antenv.axon_hooks"] = mod


def _dep(from_inst, to_inst, reason):
    from concourse.tile import add_dep_helper
    a = getattr(from_inst, "ins", from_inst)
    b = getattr(to_inst, "ins", to_inst)
    add_dep_helper(a, b, reason=reason)


def _build_nc(groups, S):
    _import_concourse()
    import concourse.bass as bass
    import concourse.tile as tile
    import concourse.tile_sem_assignment as _tsa
    from concourse import mybir

    # One DMAHW bookkeeping lane: HWDGE transfers then share a FIFO proc, so
    # DMA-vs-DMA ordering (slot WAW) needs no extra sync wait — ISA structs
    # carry at most one wait each.
    _tsa.NUM_HWDGE_SEMS = 1

    f32 = mybir.dt.float32
    bf16 = mybir.dt.bfloat16
    AF = mybir.ActivationFunctionType
    nc = bass.Bass()

    feats_d = nc.dram_tensor("feats", [18, S], bf16, kind="ExternalInput")
    xnode_d = nc.dram_tensor("xnode", [3, NCW], bf16, kind="ExternalInput")
    wpack_d = nc.dram_tensor("wpack", [128, WCOL], bf16, kind="ExternalInput")
    bpack_d = nc.dram_tensor("bpack", [128, BCOL], f32, kind="ExternalInput")
    out_d = nc.dram_tensor("out", [64, NCW], f32, kind="ExternalOutput")

    n_sup = (S + SUP - 1) // SUP

    with tile.TileContext(nc) as tc:
        with (
            tc.tile_pool(name="const", bufs=1) as cpool,
            tc.tile_pool(name="aggr", bufs=1) as apool,
            tc.tile_pool(name="feats", bufs=2) as fpool,
            tc.tile_pool(name="re", bufs=2) as repool,
            tc.tile_pool(name="gwork", bufs=1) as gpool,
            tc.tile_pool(name="psum_z", bufs=2, space="PSUM") as pz,
            tc.tile_pool(name="psum_m", bufs=2, space="PSUM") as pm,
        ):
            wsb = cpool.tile([128, WCOL], bf16, name="wsb")
            wdma = nc.sync.dma_start(wsb[:], wpack_d[:])
            bsb = cpool.tile([128, BCOL], f32, name="bsb")
            bdma = nc.sync.dma_start(bsb[:], bpack_d[:])
            w = {name: wsb[0:p, c0:c0 + cn]
                 for name, (p, c0, cn) in WSLOTS.items()}
            w.update({name: bsb[0:p, c0:c0 + cn]
                      for name, (p, c0, cn) in BSLOTS.items()})
            # ACT-side absorber: observe the bias DMA once so the first
            # bias-consuming activation doesn't need a second wait.
            tabs = cpool.tile([1, 1], f32, name="tabs")
            nc.scalar.activation(tabs[:], bsb[0:1, 0:1], AF.Copy)

            aggr = apool.tile([128, NCW], f32)

            # Matmult instructions can carry exactly one hardware sync wait;
            # a tiny absorber matmul observes the weights DMA so later
            # matmuls never need a second wait for it.
            scratch = pz.tile([128, GRP], f32, tag="zb", name="scratch")
            nc.tensor.matmul(scratch[0:1, 0:1], wsb[0:1, 0:1], wsb[0:1, 0:1],
                             start=True, stop=True)

            # Wait-absorber micro-ops: every ISA struct carries at most ONE
            # sync wait, so secondary dependencies are pre-observed by tiny
            # ops on the same engine/queue, ordered before the real op.
            vscr = cpool.tile([1, len(groups) + 4], f32, name="vscr")
            ascr = cpool.tile([1, NCW // TILE + 2], f32, name="ascr")
            ascr2 = cpool.tile([1, NCW // TILE + 2], f32, name="ascr2")
            # DVE-side absorber: observe the weights DMA once so DVE micro-
            # copies sourced from wsb need no DMA wait of their own.
            tvd0 = nc.vector.tensor_copy(vscr[0:1, len(groups) + 1:
                                              len(groups) + 2], bsb[0:1, 0:1])
            _dep(tvd0, bdma, "DVE observes bias DMA")

            sup_tiles = []
            sup_dmas = []
            for i in range(n_sup):
                cols = min(SUP, S - i * SUP)
                st = fpool.tile([18, SUP], bf16, tag="feats_sup")
                d = nc.sync.dma_start(st[:, :cols],
                                      feats_d[:, i * SUP:i * SUP + cols])
                sup_tiles.append(st)
                sup_dmas.append(d)

            runmax = []          # per-group reducer instruction
            for gi, (c0, a0, fd, first) in enumerate(groups):
                st = sup_tiles[c0 // SUP]
                fo = c0 % SUP
                fa = st[:, fo:fo + fd]
                zb = pz.tile([128, fd], f32, tag="zb")
                ms = pm.tile([128, fd], f32, tag="ms")
                mm_zb = [nc.tensor.matmul(zb[:, o:o + TILE], w["w1n"],
                                          fa[:, o:o + TILE],
                                          start=True, stop=True)
                         for o in range(0, fd, TILE)]
                # redistribute waits: the DVE release of this group's ms slot
                # lands on the second zb matmul (wait-free) instead of the
                # first ms matmul (which already carries a PE self-wait).
                if gi >= 2:
                    _dep(mm_zb[1], runmax[gi - 2], "ms-slot release via zb twin")
                # a new feats superblock must land before the NEXT group that
                # reads it; its wait goes on this group's e-twin (below).
                for o in range(0, fd, TILE):
                    nc.tensor.matmul(ms[:, o:o + TILE], w["w12"],
                                     fa[:, o:o + TILE], start=True, stop=False)
                r = repool.tile([128, fd], bf16, tag="r")
                e = repool.tile([128, fd], bf16, tag="e")
                nc.scalar.activation(r[:], zb[:], AF.Relu,
                                     bias=w["nbias1"], scale=1.0)
                nc.scalar.activation(e[:], r[:], AF.Exp, scale=-1.0)
                for o in range(0, fd, TILE):
                    nc.tensor.matmul(ms[:, o:o + TILE], w["w2"],
                                     r[:, o:o + TILE], start=False, stop=False)
                mm_e = [nc.tensor.matmul(ms[:, o:o + TILE], w["w2"],
                                         e[:, o:o + TILE],
                                         start=False, stop=(o + TILE >= fd))
                        for o in range(0, fd, TILE)]
                nxt = (c0 + fd) // SUP
                if nxt > c0 // SUP and nxt < n_sup:
                    _dep(mm_e[1], sup_dmas[nxt], "sup prefetch via e twin")
                # DVE pre-observes the msg matmuls' completion so the reducer
                # carries only its own in-order RAW wait.
                tv = nc.vector.tensor_copy(vscr[0:1, gi:gi + 1],
                                           bsb[0:1, 0:1])
                _dep(tv, mm_e[1], "absorb reducer PE wait")
                dst_ap = aggr[:, a0:a0 + fd]
                if first:
                    rm = nc.vector.tensor_copy(dst_ap, ms[:])
                else:
                    rm = nc.vector.tensor_max(dst_ap, dst_ap, ms[:])
                _dep(rm, tv, "order after absorber")
                runmax.append(rm)
                last_mm = mm_e[1]
                zb_last = zb

            # ---- node phase ----
            # TensorTensor needs equal base partitions for SBUF inputs:
            # DMA-move the odd-rounds half (partitions 64-127) down to 0-63.
            ah = gpool.tile([64, NCW], f32, tag="ah")
            ahdma = nc.sync.dma_start(ah[:], aggr[64:128, :])
            tva = nc.vector.tensor_copy(vscr[0:1, len(groups):len(groups) + 1],
                                        bsb[0:1, 0:1])
            _dep(tva, ahdma, "absorb aggr-move DMA wait")
            fold = nc.vector.tensor_max(ah[:], aggr[0:64, :], ah[:])
            _dep(fold, tva, "order after absorber")
            u_in = gpool.tile([67, NCW], bf16, tag="u_in")
            urelu = nc.scalar.activation(u_in[0:64, :], ah[:], AF.Relu,
                                         bias=w["cbias"], scale=1.0)
            xdma = nc.sync.dma_start(u_in[64:67, :], xnode_d[:])
            out_sb = gpool.tile([64, NCW], f32, tag="out_sb")

            # Absorber chain: tiny matmuls into the last group's dead zb
            # tile (claiming no new PSUM slot) make PE observe the final
            # reducer's DVE tick, the xnode DMA, and the u_in relu, so each
            # g-phase matmul keeps at most one hardware wait (its own PSUM
            # slot-reuse self-wait).
            scr2 = zb_last
            t2 = nc.tensor.matmul(scr2[0:1, 0:1], wsb[0:1, 0:1],
                                  wsb[0:1, 0:1], start=True, stop=False)
            _dep(t2, runmax[-1], "observe final reducer DVE tick")
            t3 = nc.tensor.matmul(scr2[0:1, 0:1], wsb[0:1, 0:1],
                                  wsb[0:1, 0:1], start=False, stop=False)
            _dep(t3, xdma, "observe xnode DMA")
            t4 = nc.tensor.matmul(scr2[0:1, 0:1], wsb[0:1, 0:1],
                                  wsb[0:1, 0:1], start=False, stop=True)
            _dep(t4, urelu, "observe u_in relu ACT tick")

            for i in range(NCW // TILE):
                ui = u_in[:, i * TILE:(i + 1) * TILE]
                zg = pz.tile([64, TILE], f32, tag="zb")
                o2 = pm.tile([64, TILE], f32, tag="ms")
                mm_zg = nc.tensor.matmul(zg[:], w["g1n"], ui,
                                         start=True, stop=True)
                nc.tensor.matmul(o2[:], w["g12"], ui, start=True, stop=False)
                rg = repool.tile([64, TILE], bf16, tag="r")
                eg = repool.tile([64, TILE], bf16, tag="e")
                # ACT pre-observes the g1 matmul so rg keeps only its own
                # slot-WAW wait
                tag_ = nc.scalar.activation(ascr2[0:1, i:i + 1], bsb[0:1, 0:1],
                                            AF.Copy)
                _dep(tag_, mm_zg, "absorb rg PE wait")
                rgi = nc.scalar.activation(rg[:], zg[:], AF.Relu,
                                           bias=w["nbias_g1"], scale=1.0)
                _dep(rgi, tag_, "order after absorber")
                nc.scalar.activation(eg[:], rg[:], AF.Exp, scale=-1.0)
                nc.tensor.matmul(o2[:], w["g2"], rg[:], start=False,
                                 stop=False)
                nc.tensor.matmul(o2[:], w["g2"], eg[:], start=False,
                                 stop=True)
                rf = repool.tile([64, TILE], f32, tag="rf")
                ef = repool.tile([64, TILE], f32, tag="ef")
                vf = repool.tile([64, TILE], f32, tag="vf")
                rf_act_deps = []
                if i >= 2:
                    # ACT pre-observes the combiner's DVE tick (releases the
                    # rf/ef/vf slots of tile i-2)
                    ta = nc.scalar.activation(ascr[0:1, i:i + 1],
                                              bsb[0:1, 0:1], AF.Copy)
                    _dep(ta, stt_prev2, "absorb final-combine DVE wait")
                    rf_act_deps.append(ta)
                rfi = nc.scalar.activation(rf[:], o2[:], AF.Relu,
                                           bias=w["nbias_gf"], scale=-1.0)
                for ta_ in rf_act_deps:
                    _dep(rfi, ta_, "order after absorber")
                nc.scalar.activation(ef[:], rf[:], AF.Exp, scale=-1.0)
                nc.scalar.activation(vf[:], o2[:], AF.Relu,
                                     bias=w["pbias_gf"], scale=1.0)
                stt = nc.vector.scalar_tensor_tensor(
                    out_sb[:, i * TILE:(i + 1) * TILE], vf[:], -1.0, ef[:],
                    op0=mybir.AluOpType.add, op1=mybir.AluOpType.add)
                if i >= 1:
                    stt_prev2 = stt_prev
                stt_prev = stt

            nc.sync.dma_start(out_d[:], out_sb[:])

    _prune_waits(nc)
    return nc


def _prune_waits(nc):
    """ISA structs carry at most one sync wait. Drop provably-redundant
    waits Tile emitted:

    1. same-engine self-waits on compute instructions other than Matmult:
       ACT/DVE/Pool queues are strict FIFO and each op fully drains before
       the next issues, so an earlier instruction on the same engine is
       always complete; the dependency the wait encodes is enforced by
       program order (the earlier instruction itself blocks the queue while
       ITS waits are pending).  PE kept: consecutive matmuls overlap
       fill/drain in the array.
    2. DMA-vs-DMA ordering waits on transfers that also carry a compute
       wait: in this program's dataflow the compute dependency is on
       readers of the slot's previous contents (or on consumers downstream
       of every earlier conflicting transfer), and a completed read implies
       the producing DMA completed.
    """
    n1 = n2 = 0
    for b in nc.m.functions[0].blocks:
        for i in b.instructions:
            si = i.sync_info
            if si is None or not si.on_wait or len(si.on_wait) < 2:
                continue
            nm = type(i).__name__
            waits = list(si.on_wait)
            if nm == "InstDrain":
                # kernel-tail drain: every engine's last instruction is
                # observed (transitively) by the final output DMA, so the
                # single DMAHW wait subsumes the engine waits here.
                dma_w = [x for x in waits if x.ant_name.startswith("DMAHW")]
                if dma_w:
                    si.on_wait = dma_w[-1:]
                else:
                    si.on_wait = waits[-1:]
                continue
            if nm == "InstDMACopy":
                if any(not x.ant_name.startswith("DMAHW") and
                       not x.ant_name.startswith("DMASW") for x in waits):
                    kept = [x for x in waits
                            if not (x.ant_name.startswith("DMAHW") or
                                    x.ant_name.startswith("DMASW"))]
                    n2 += len(waits) - len(kept)
                    waits = kept
            else:
                # Matmult included: matmuls complete in pc order (start AND
                # end monotone), and every PSUM slot-reuse WAW in this
                # program is >=8 matmuls distant, far beyond the fill/drain
                # overlap of adjacent instructions.
                own = str(i.engine).split(".")[-1]
                kept = [x for x in waits
                        if x.ant_name.rsplit("_", 1)[0] != own]
                if len(kept) < len(waits):
                    n1 += len(waits) - len(kept)
                    waits = kept
            si.on_wait = waits
    return n1, n2


# --------------------------------------------------------------------------
# entry points
# --------------------------------------------------------------------------

def _prepare(x, pos, edge_index, f_w1, f_b1, f_w2, f_b2,
             g_w1, g_b1, g_w2, g_b2):
    x = np.asarray(x, F32)
    pos = np.asarray(pos, F32)
    src = np.asarray(edge_index[0]).astype(np.int64)
    dst = np.asarray(edge_index[1]).astype(np.int64)
    cores = _core_layouts(edge_index)
    tiles, groups, S = _tile_plan(cores)
    S_pad = ((S + SUP - 1) // SUP) * SUP
    packs = []
    for c, core in enumerate(cores):
        feats, xnode = _pack_core(core, tiles, S_pad, x, pos, src, dst)
        xnode[:, :NCN] = x[core["order"] + c * NCN].T
        packs.append((feats, xnode))
    w = _weights(np.asarray(f_w1, F32), np.asarray(f_b1, F32),
                 np.asarray(f_w2, F32), np.asarray(f_b2, F32),
                 np.asarray(g_w1, F32), np.asarray(g_b1, F32),
                 np.asarray(g_w2, F32), np.asarray(g_b2, F32))
    return cores, groups, S_pad, packs, w


def _finalize(results, cores, x, g_w1, g_b1, g_w2, g_b2):
    """results: list of [64, NCW] per core -> full [N, 64] output."""
    out = np.empty((N, 64), dtype=F32)
    for c, core in enumerate(cores):
        out[core["order"] + c * NCN] = results[c][:, :NCN].T
    empties = np.concatenate([c["empty"] for c in cores])
    if empties.size:
        def celu(v):
            return np.maximum(v, 0) + np.minimum(0, np.expm1(np.minimum(v, 0)))
        u_in = np.concatenate(
            [np.zeros((empties.size, 64), F32), x[empties]], axis=1)
        u = celu(u_in @ g_w1 + g_b1)
        out[empties] = celu(u @ g_w2 + g_b2).astype(F32)
    return out


def kernel(x, pos, edge_index, f_w1, f_b1, f_w2, f_b2,
           g_w1, g_b1, g_w2, g_b2, _debug_numpy=False, _trace=False):
    x = np.asarray(x, F32)
    pos = np.asarray(pos, F32)
    cores, groups, S_pad, packs, w = _prepare(
        x, pos, edge_index, f_w1, f_b1, f_w2, f_b2, g_w1, g_b1, g_w2, g_b2)

    if _debug_numpy:
        results = [_numpy_device(f, xn, w, groups) for (f, xn) in packs]
        return _finalize(results, cores, x, np.asarray(g_w1, F32),
                         np.asarray(g_b1, F32), np.asarray(g_w2, F32),
                         np.asarray(g_b2, F32))

    _import_concourse()
    run_kwargs = {}
    if _trace:
        _install_ntff_shim()
        import concourse.bass_utils as _bu
        _bu.upload_artifacts = lambda tmpdir: f"file://{tmpdir}"
        import tempfile
        trace_dir = tempfile.mkdtemp(prefix="bass_trace_")
        run_kwargs = dict(tmpdir=trace_dir)
        kernel._last_trace_dir = trace_dir
    from concourse.bass_utils import run_bass_kernel_spmd

    import ml_dtypes
    bf = ml_dtypes.bfloat16
    nc = _build_nc(groups, S_pad)
    in_maps = [{"feats": feats.astype(bf), "xnode": xnode.astype(bf),
                "wpack": w["wpack"], "bpack": w["bpack"]}
               for (feats, xnode) in packs]
    res = run_bass_kernel_spmd(nc, in_maps, list(range(CORES)), trace=_trace,
                               **run_kwargs)
    results = [res.results[c]["out"] for c in range(CORES)]
    out = _finalize(results, cores, x, np.asarray(g_w1, F32),
                    np.asarray(g_b1, F32), np.asarray(g_w2, F32),
                    np.asarray(g_b2, F32))
    if _trace:
        kernel._last_exec_time_ns = res.exec_time_ns
        kernel._last_mean_exec_time_ns = res.mean_exec_time_ns
    return out



# revision 10
# speedup vs baseline: 1.2056x; 1.2056x over previous
"""Trainium2 Bass kernel for a GNN message-passing layer.

Reference semantics (per edge e = (src j, dst i)):
    m_in  = [x_j, pos_j - pos_i]                 # [E, 6]
    h     = celu(m_in @ f_w1 + f_b1)             # [E, 64]
    msg   = relu(h @ f_w2 + f_b2)                # [E, 64]
    aggr  = segment_max(msg, dst, N); empty -> 0 # [N, 64]
    u     = celu([aggr, x] @ g_w1 + g_b1)
    out   = celu(u @ g_w2 + g_b2)                # [N, 64]

Sharding: nodes split into 8 contiguous ranges (6250/core); each core gets the
edges whose dst is in its range, so segment-max is local.  Host does
index-only work (degree-sort, round layout, gather); device does every FLOP.

Device program (v2): celu decomposed as celu(z) = relu(-z) + exp(-relu(-z))
+ z - 1.  Per 1024-column group (2 edges stacked per column):
  zb = w9@f (PSUM), then either
    A-path: r = ACT.Relu(-zb-b1), e = ACT.Exp(-r); ms += w2@r + w2@e
    D-path: m = DVE.ts(zb+b1 min 0) (= -r), e = ACT.Exp(m); ms += (-w2)@m + w2@e
  ms also accumulates w12@f (the linear z term), then DVE tensor-max into a
  bf16 running aggregate (relu+bias deferred past the max).
The PE stream is software-pipelined depth-2 (w2-streams of group g run while
zb of g+2 and ms-init of g+1 are computed) so the tensor engine never waits
on ACT; a gap-free warmup burst un-throttles the PE HAM clock gate
(1.2 -> 2.4 GHz) at kernel start and keep-warm dummies span the node-phase
lead-in.
"""

import math
import os
import sys

import numpy as np

N = 50000
E = 1600000
CORES = 8
NCN = N // CORES            # nodes per core
TILE = 512                  # fp32 matmul moving free dim / one PSUM bank
GRP = 1024                  # group width (columns) = 2 tiles
SUP = 4096                  # feats DMA staging superblock (columns) = 4 groups
F32 = np.float32
DPAT = 3                    # every DPAT-th group takes the DVE (m) path


# --------------------------------------------------------------------------
# host-side layout (index work only)
# --------------------------------------------------------------------------

def _core_layouts(edge_index):
    """Per-core node ordering + degree-sorted CSR of local edges."""
    dst = np.asarray(edge_index[1])
    cores = []
    for c in range(CORES):
        lo, hi = c * NCN, (c + 1) * NCN
        eids = np.nonzero((dst >= lo) & (dst < hi))[0]
        ldst = (dst[eids] - lo).astype(np.int64)
        deg = np.bincount(ldst, minlength=NCN)
        order = np.argsort(-deg, kind="stable")         # node ranks
        rank = np.empty(NCN, np.int64)
        rank[order] = np.arange(NCN)
        perm = np.argsort(rank[ldst], kind="stable")
        es = eids[perm]                                  # edges sorted by rank
        deg_s = deg[order]
        row_start = np.zeros(NCN + 1, np.int64)
        np.cumsum(deg_s, out=row_start[1:])
        cores.append(dict(es=es, deg_s=deg_s, row_start=row_start,
                          order=order, empty=order[deg_s == 0] + lo))
    return cores


def _tile_plan(cores):
    """Shared (SPMD-uniform) tile plan at 512-column granularity.

    tiles: list of (pair_round t, node_block k); tile covers node ranks
    [512k, 512k+512) at rounds (2t, 2t+1).  Flat consecutive pairs of tiles
    form 1024-column groups (groups may straddle rounds; the aggregate-max
    is per-tile anyway).
    """
    rmax = max(int(c["deg_s"][0]) for c in cores)
    n_pairs = (rmax + 1) // 2
    tiles = []
    for t in range(n_pairs):
        w = max(int(np.searchsorted(-c["deg_s"], -(2 * t), side="left"))
                for c in cores)      # max over cores of #nodes with deg > 2t
        if t == 0:
            w = NCN                  # every aggr column gets initialized
        for k in range(max(1, (w + TILE - 1) // TILE)):
            tiles.append((t, k))
    if len(tiles) % 2:
        assert tiles[-1][0] > 0
        tiles.append(tiles[-1])      # dup: max is idempotent, not first-touch
    S = TILE * len(tiles)
    ncw = TILE * ((NCN + TILE - 1) // TILE)
    return tiles, S, ncw


def _pack_core(core, tiles, S, ncw, x, pos, src, dst):
    """Build one core's slot->edge assignment and gather features."""
    es, deg_s, row_start = core["es"], core["deg_s"], core["row_start"]
    ncols = len(tiles) * TILE
    nvec = np.tile(np.arange(TILE, dtype=np.int64), len(tiles))  # col in tile
    kvec = np.repeat([k for (_, k) in tiles], TILE)
    tvec = np.repeat([t for (t, _) in tiles], TILE)
    node = kvec * TILE + nvec                    # node rank targeted by column

    safe_node = np.minimum(node, NCN - 1)
    ecap = len(es) - 1
    first_edge = es[np.minimum(row_start[safe_node], ecap)]  # dup fallback
    bad = (node >= NCN) | (deg_s[safe_node] == 0)
    first_edge = np.where(bad, es[0], first_edge)

    def round_edges(r):
        has = (~bad) & (deg_s[safe_node] > r)
        idx = np.minimum(row_start[safe_node] + np.where(has, r, 0), ecap)
        return np.where(has, es[idx], first_edge)

    a_e = round_edges(2 * tvec)        # vectorized: r differs per column
    b_e = round_edges(2 * tvec + 1)

    feats = np.empty((18, S), dtype=F32)
    for half, eids in ((0, a_e), (9, b_e)):
        s, d = src[eids], dst[eids]
        feats[half + 0:half + 3, :ncols] = x[s].T
        feats[half + 3:half + 6, :ncols] = pos[s].T
        feats[half + 6:half + 9, :ncols] = pos[d].T
    if ncols < S:
        feats[:, ncols:] = 0.0

    xnode = np.zeros((3, ncw), dtype=F32)
    xnode[:, :NCN] = x[core["order"] + 0].T      # caller adds core offset
    return feats, xnode


# column layouts of the packed weight tensors (bf16 matmul operands; PE runs
# fp32 at 1/4 rate, bf16 streams 1 col/cycle with f32 PSUM accumulation)
WSLOTS = dict(w1n=(18, 0, 128), w12=(18, 128, 128), w2p=(128, 256, 128),
              w2n=(128, 384, 128), g1n=(67, 512, 64), g12=(67, 576, 64),
              g2=(64, 640, 64))
WCOL = 704
BSLOTS = dict(nbias1=(128, 0, 1), pbias1=(128, 1, 1), cbias=(64, 2, 1),
              ngb1=(64, 3, 1), gbias=(64, 4, 1))
BCOL = 8


def _weights(f_w1, f_b1, f_w2, f_b2, g_w1, g_b1, g_w2, g_b2):
    w9 = np.concatenate([f_w1[0:3], f_w1[3:6], -f_w1[3:6]], axis=0)  # [9,64]
    blk = lambda m: np.block([[m, np.zeros_like(m)], [np.zeros_like(m), m]])
    cbias = (f_b1 @ f_w2 - f_w2.sum(axis=0) + f_b2).astype(F32)       # [64]
    gbias = (g_b1 @ g_w2 - g_w2.sum(axis=0) + g_b2).astype(F32)       # [64]
    w = dict(
        w1n=blk(w9).astype(F32),             # [18,128]  (zb = +z1)
        w12=blk(w9 @ f_w2).astype(F32),      # [18,128]
        w2p=blk(f_w2).astype(F32),           # [128,128]
        w2n=blk(-f_w2).astype(F32),          # [128,128]
        g1n=g_w1.astype(F32),                # [67,64]
        g12=(g_w1 @ g_w2).astype(F32),       # [67,64]
        g2=g_w2.astype(F32),                 # [64,64]
        nbias1=np.tile(-f_b1, 2).astype(F32).reshape(128, 1),
        pbias1=np.tile(f_b1, 2).astype(F32).reshape(128, 1),
        cbias=cbias.reshape(64, 1),
        ngb1=(-g_b1).astype(F32).reshape(64, 1),
        gbias=gbias.reshape(64, 1),
    )
    import ml_dtypes
    wpack = np.zeros((128, WCOL), dtype=ml_dtypes.bfloat16)
    for name, (p, c0, cn) in WSLOTS.items():
        wpack[:p, c0:c0 + cn] = w[name]
    bpack = np.zeros((128, BCOL), dtype=F32)
    for name, (p, c0, cn) in BSLOTS.items():
        bpack[:p, c0:c0 + cn] = w[name]
    w["wpack"] = wpack
    w["bpack"] = bpack
    return w


def _bf(v):
    import ml_dtypes
    return np.asarray(v).astype(ml_dtypes.bfloat16).astype(F32)


# --------------------------------------------------------------------------
# numpy model of the device program (for validation; mimics bf16 rounding)
# --------------------------------------------------------------------------

def _numpy_device(feats, xnode, w, tiles, ncw):
    G = len(tiles) // 2
    aggr = np.zeros((128, ncw), dtype=F32)
    for g in range(G):
        f = _bf(feats[:, g * GRP:(g + 1) * GRP])
        zb = w["w1n"].T @ f                                  # +z1
        dve_path = (g % DPAT) == (DPAT - 1)
        if dve_path:
            m = _bf(np.minimum(zb + w["pbias1"], 0))
            e = _bf(np.exp(m))
            ms = w["w12"].T @ f + w["w2n"].T @ m + w["w2p"].T @ e
        else:
            r = _bf(np.maximum(-zb + w["nbias1"], 0))
            e = _bf(np.exp(-r))
            ms = w["w12"].T @ f + w["w2p"].T @ r + w["w2p"].T @ e
        for j in (0, 1):
            t, k = tiles[2 * g + j]
            dst = aggr[:, k * TILE:(k + 1) * TILE]
            src = _bf(ms[:, j * TILE:(j + 1) * TILE])
            if t == 0:
                dst[:] = src
            else:
                np.maximum(dst, src, out=dst)
    a64 = np.maximum(aggr[0:64], aggr[64:128])
    u_in = np.empty((67, ncw), dtype=F32)
    u_in[0:64] = _bf(np.maximum(a64 + w["cbias"], 0))
    u_in[64:67] = _bf(xnode)
    out = np.empty((64, ncw), dtype=F32)
    for i in range(ncw // TILE):
        ui = u_in[:, i * TILE:(i + 1) * TILE]
        zg = w["g1n"].T @ ui
        rg = _bf(np.maximum(-zg + w["ngb1"], 0))
        eg = _bf(np.exp(-rg))
        o2 = w["g12"].T @ ui + w["g2"].T @ rg + w["g2"].T @ eg
        vf = _bf(np.maximum(o2 + w["gbias"], 0))
        mf = _bf(np.minimum(o2 + w["gbias"], 0))
        ef = _bf(np.exp(mf))
        out[:, i * TILE:(i + 1) * TILE] = _bf((ef - 1.0) + vf)
    return out        # [64, ncw] (bf16-rounded values)


# --------------------------------------------------------------------------
# bass program
# --------------------------------------------------------------------------

def _import_concourse():
    try:
        import concourse.bass  # noqa: F401
    except ImportError:
        sys.path.insert(0, "/opt/trn_rl_repo")


def _install_ntff_shim():
    """Provide antenv.axon_hooks (missing in this image) so that
    run_bass_kernel_spmd(trace=True) can capture NTFF profiles."""
    import contextlib
    import ctypes
    import types

    if "antenv.axon_hooks" in sys.modules:
        return
    so_path = "/opt/axon/libaxon_pjrt.so"
    if not os.path.exists(so_path):
        return
    lib = ctypes.CDLL(so_path)
    if not hasattr(lib, "axon_start_nrt_profile"):
        return
    lib.axon_start_nrt_profile.argtypes = [ctypes.POINTER(ctypes.c_int64),
                                           ctypes.c_size_t]
    lib.axon_start_nrt_profile.restype = ctypes.c_int64
    lib.axon_stop_nrt_profile.argtypes = [ctypes.c_char_p]
    lib.axon_stop_nrt_profile.restype = ctypes.c_int64

    @contextlib.contextmanager
    def _hook(output_dir, device_ids):
        import jax
        jax.devices()
        if device_ids:
            ids = (ctypes.c_int64 * len(device_ids))(*device_ids)
            rc = lib.axon_start_nrt_profile(ids, len(device_ids))
        else:
            rc = lib.axon_start_nrt_profile(None, 0)
        if rc != 0:
            raise RuntimeError(f"axon_start_nrt_profile rc={rc}")
        try:
            yield
        finally:
            n = lib.axon_stop_nrt_profile(str(output_dir).encode())
            print(f"ntff profile: {n} file(s) -> {output_dir}",
                  file=sys.stderr)

    mod = types.ModuleType("antenv.axon_hooks")
    mod.get_axon_ntff_profile_hook = lambda: _hook
    mod.set_axon_ntff_profile_hook = lambda h: None
    sys.modules["antenv.axon_hooks"] = mod


def _dep(from_inst, to_inst, reason):
    from concourse.tile import add_dep_helper
    a = getattr(from_inst, "ins", from_inst)
    b = getattr(to_inst, "ins", to_inst)
    add_dep_helper(a, b, reason=reason)


def _build_nc(tiles, S, ncw):
    _import_concourse()
    import concourse.bass as bass
    import concourse.tile as tile
    import concourse.tile_sem_assignment as _tsa
    from concourse import mybir

    # One DMAHW bookkeeping lane: HWDGE transfers share a FIFO proc, so
    # DMA-vs-DMA ordering (slot WAW) needs no extra sync wait.
    _tsa.NUM_HWDGE_SEMS = 1

    f32 = mybir.dt.float32
    bf16 = mybir.dt.bfloat16
    AF = mybir.ActivationFunctionType
    ALU = mybir.AluOpType
    nc = bass.Bass()

    G = len(tiles) // 2
    S_pad = ((S + SUP - 1) // SUP) * SUP
    n_sup = S_pad // SUP
    n_nt = ncw // TILE                       # node tiles

    feats_d = nc.dram_tensor("feats", [18, S_pad], bf16, kind="ExternalInput")
    xnode_d = nc.dram_tensor("xnode", [3, ncw], bf16, kind="ExternalInput")
    wpack_d = nc.dram_tensor("wpack", [128, WCOL], bf16, kind="ExternalInput")
    bpack_d = nc.dram_tensor("bpack", [128, BCOL], f32, kind="ExternalInput")
    out_d = nc.dram_tensor("out", [64, ncw], bf16, kind="ExternalOutput")

    # node-phase lead-in chunks (4 tiles each) and the edge-group after which
    # each chunk's aggr columns are final (chunk 0 = blocks 0-3 is last)
    n_ck = (n_nt + 3) // 4
    ck_last = []
    for c in range(n_ck):
        blocks = set(range(4 * c, min(4 * c + 4, n_nt)))
        last = 0
        for j, (t, k) in enumerate(tiles):
            if k in blocks:
                last = j // 2
        ck_last.append(last)

    with tile.TileContext(nc) as tc:
        with (
            tc.tile_pool(name="const", bufs=1) as cpool,
            tc.tile_pool(name="aggr", bufs=1) as apool,
            tc.tile_pool(name="feats", bufs=2) as fpool,
            tc.tile_pool(name="re", bufs=2) as repool,
            tc.tile_pool(name="gwork", bufs=1) as gpool,
            tc.tile_pool(name="nre", bufs=2) as nrepool,
            tc.tile_pool(name="psum_z", bufs=2, space="PSUM") as pz,
            tc.tile_pool(name="psum_m", bufs=2, space="PSUM") as pm,
        ):
            wsb = cpool.tile([128, WCOL], bf16, name="wsb")
            wdma = nc.sync.dma_start(wsb[:], wpack_d[:])
            bsb = cpool.tile([128, BCOL], f32, name="bsb")
            bdma = nc.sync.dma_start(bsb[:], bpack_d[:])
            w = {name: wsb[0:p, c0:c0 + cn]
                 for name, (p, c0, cn) in WSLOTS.items()}
            w.update({name: bsb[0:p, c0:c0 + cn]
                      for name, (p, c0, cn) in BSLOTS.items()})
            # ACT-side absorber: observe the bias DMA once.
            tabs = cpool.tile([1, 8], f32, name="tabs")
            ta0 = nc.scalar.activation(tabs[0:1, 0:1], bsb[0:1, 0:1], AF.Copy)
            _dep(ta0, bdma, "ACT observes bias DMA")
            # DVE-side absorber for the bias DMA.
            vscr = cpool.tile([1, 8], f32, name="vscr")
            tv0 = nc.vector.tensor_copy(vscr[0:1, 0:1], bsb[0:1, 0:1])
            _dep(tv0, bdma, "DVE observes bias DMA")

            aggr = apool.tile([128, ncw], bf16)
            u_in = gpool.tile([67, ncw], bf16, tag="u_in")
            ah = gpool.tile([64, ncw], bf16, tag="ah")
            out_sb = gpool.tile([64, ncw], bf16, tag="out_sb")

            # ---- HAM warmup burst: gap-free dummy matmuls un-throttle the
            # PE clock gate (4096-cycle activity window) before real work.
            warm = pz.tile([128, GRP], f32, tag="zb", name="warm")
            for i in range(14):
                nc.tensor.matmul(warm[:, 0:TILE], wsb[:, 0:128],
                                 wsb[:, 0:TILE], start=True, stop=True)

            # ---- feats superblock staging
            sup_tiles = [None] * n_sup
            sup_dmas = [None] * n_sup
            for i in range(min(2, n_sup)):
                st = fpool.tile([18, SUP], bf16, tag="feats_sup")
                sup_dmas[i] = nc.sync.dma_start(
                    st[:], feats_d[:, i * SUP:(i + 1) * SUP])
                sup_tiles[i] = st

            def fcols(g):
                c0 = g * GRP
                st = sup_tiles[c0 // SUP]
                fo = c0 % SUP
                return st[:, fo:fo + GRP]

            def emit_zb(g):
                """zb(g) = w9 @ feats(g)  (2 x 512-col matmuls)."""
                zbt = pz.tile([128, GRP], f32, tag="zb")
                fa = fcols(g)
                mm = [nc.tensor.matmul(zbt[:, o:o + TILE], w["w1n"],
                                       fa[:, o:o + TILE], start=True,
                                       stop=True) for o in (0, TILE)]
                return zbt, mm

            def emit_msinit(g):
                mst = pm.tile([128, GRP], f32, tag="ms")
                fa = fcols(g)
                mm = [nc.tensor.matmul(mst[:, o:o + TILE], w["w12"],
                                       fa[:, o:o + TILE], start=True,
                                       stop=False) for o in (0, TILE)]
                return mst, mm

            def emit_re(g, zbt, zbmm):
                """ACT/DVE nonlinear ops for group g (runs one iter ahead)."""
                dve_path = (g % DPAT) == (DPAT - 1)
                rm = repool.tile([128, GRP], bf16, tag="rm")
                et = repool.tile([128, GRP], bf16, tag="e")
                if dve_path:
                    # DVE pre-observes the zb matmuls so the tensor_scalar
                    # keeps a single sync wait (its slot-reuse WAR)
                    va = nc.vector.tensor_copy(vscr[0:1, 2:3], bsb[0:1, 0:1])
                    _dep(va, zbmm[1], "DVE pre-observes zb")
                    mo = nc.vector.tensor_scalar(
                        rm[:], zbt[:], w["pbias1"], 0.0, ALU.add, ALU.min)
                    _dep(mo, va, "order after absorber")
                    # absorber: ACT observes the DVE op so the Exp carries
                    # only its own slot-reuse PE wait
                    tae = nc.scalar.activation(tabs[0:1, 1:2], bsb[0:1, 0:1],
                                               AF.Copy)
                    _dep(tae, mo, "ACT absorbs DVE m dep")
                    ee = nc.scalar.activation(et[:], rm[:], AF.Exp, scale=1.0)
                    _dep(ee, tae, "order after absorber")
                else:
                    nc.scalar.activation(rm[:], zbt[:], AF.Relu,
                                         bias=w["nbias1"], scale=-1.0)
                    nc.scalar.activation(et[:], rm[:], AF.Exp, scale=-1.0)
                return rm, et, dve_path

            def emit_w2(g, mst, rm, et, dve_path):
                """w2-streams for group g into ms(g); returns the e-MMs."""
                wrm = w["w2n"] if dve_path else w["w2p"]
                for o in (0, TILE):
                    nc.tensor.matmul(mst[:, o:o + TILE], wrm,
                                     rm[:, o:o + TILE], start=False,
                                     stop=False)
                mm_e = []
                for o in (0, TILE):
                    mm_e.append(nc.tensor.matmul(
                        mst[:, o:o + TILE], w["w2p"], et[:, o:o + TILE],
                        start=False, stop=True))
                return mm_e

            def emit_aggmax(g, mst, mm_e):
                outs = []
                for j in (0, 1):
                    t, k = tiles[2 * g + j]
                    dst = aggr[:, k * TILE:(k + 1) * TILE]
                    src = mst[:, j * TILE:(j + 1) * TILE]
                    if t == 0:
                        rmx = nc.vector.tensor_copy(dst, src)
                    else:
                        rmx = nc.vector.tensor_max(dst, dst, src)
                    outs.append(rmx)
                return outs

            # ---- node-phase lead-in (per 4-tile chunk): move odd-round half
            # down, fold max, relu+cbias into u_in.  Emitted as soon as the
            # chunk's aggr columns are final so it hides under the edge phase.
            def emit_chunk(c, after_dve=None):
                c0 = 4 * c * TILE
                cw = min(ncw - c0, 4 * TILE)
                d = nc.sync.dma_start(ah[:, c0:c0 + cw],
                                      aggr[64:128, c0:c0 + cw])
                tvc = nc.vector.tensor_copy(vscr[0:1, 1:2], bsb[0:1, 0:1])
                _dep(tvc, d, "DVE absorbs fold DMA dep")
                fo = nc.vector.tensor_max(ah[:, c0:c0 + cw],
                                          aggr[0:64, c0:c0 + cw],
                                          ah[:, c0:c0 + cw])
                _dep(fo, tvc, "order after absorber")
                ur = nc.scalar.activation(u_in[0:64, c0:c0 + cw],
                                          ah[:, c0:c0 + cw], AF.Relu,
                                          bias=w["cbias"], scale=1.0)
                return ur

            # =========== edge phase ===========
            zb_t = {}
            zb_mm = {}
            ms_t = {}
            re_t = {}
            zb_t[0], zb_mm[0] = emit_zb(0)
            ms_t[0], _ = emit_msinit(0)
            if G > 1:
                zb_t[1], zb_mm[1] = emit_zb(1)
            re_t[0] = emit_re(0, zb_t[0], zb_mm[0])

            chunks_done = set()
            chunk_insts = {}
            for g in range(G):
                rm, et, dve_path = re_t.pop(g)
                mm_e = emit_w2(g, ms_t[g], rm, et, dve_path)
                # if groups g+1/g+2 (read next on PE) start a new superblock,
                # absorb that DMA's wait on the wait-free e-twin
                c_nxt = (g + 2) * GRP
                if g + 2 < G and c_nxt % SUP == 0 \
                        and sup_dmas[c_nxt // SUP] is not None:
                    _dep(mm_e[1], sup_dmas[c_nxt // SUP],
                         "sup prefetch via e twin")
                agg = emit_aggmax(g, ms_t.pop(g), mm_e)
                if g + 2 < G:
                    # prefetch the superblock that group g+3 will read
                    c3 = (g + 3) * GRP
                    if g + 3 < G and c3 % SUP == 0 and c3 // SUP < n_sup \
                            and sup_tiles[c3 // SUP] is None:
                        st = fpool.tile([18, SUP], bf16, tag="feats_sup")
                        sup_dmas[c3 // SUP] = nc.sync.dma_start(
                            st[:], feats_d[:, c3:c3 + SUP])
                        sup_tiles[c3 // SUP] = st
                    zb_t[g + 2], zb_mm[g + 2] = emit_zb(g + 2)
                if g + 1 < G:
                    ms_t[g + 1], _ = emit_msinit(g + 1)
                    re_t[g + 1] = emit_re(g + 1, zb_t[g + 1], zb_mm[g + 1])
                # early node-phase chunks once their blocks are final
                for c in range(1, n_ck):
                    if c not in chunks_done and ck_last[c] == g:
                        chunks_done.add(c)
                        chunk_insts[c] = emit_chunk(c)

            xdma = nc.sync.dma_start(u_in[64:67, :], xnode_d[:])
            chunk_insts[0] = emit_chunk(0)
            for c in range(1, n_ck):
                if c not in chunks_done:
                    chunk_insts[c] = emit_chunk(c)

            # keep-warm dummies across the lead-in gap (PE idle > ~3.4us
            # would re-throttle the clock gate); kw2 also absorbs the xnode
            # DMA wait so the first node matmul keeps a single sem wait
            kw1 = nc.tensor.matmul(warm[:, 0:TILE], wsb[:, 0:128],
                                   wsb[:, 0:TILE], start=True, stop=True)
            _dep(kw1, chunk_insts[0], "keep PE warm past fold chunk 0")
            kw2 = nc.tensor.matmul(warm[:, 0:TILE], wsb[:, 0:128],
                                   wsb[:, 0:TILE], start=True, stop=True)
            _dep(kw2, xdma, "absorb xnode DMA wait")

            # =========== node phase ===========
            prev_stt = {}
            for i in range(n_nt):
                ui = u_in[:, i * TILE:(i + 1) * TILE]
                zg = pz.tile([64, TILE], f32, tag="zb")
                o2 = pm.tile([64, TILE], f32, tag="ms")
                mm_zg = nc.tensor.matmul(zg[:], w["g1n"], ui,
                                         start=True, stop=True)
                nc.tensor.matmul(o2[:], w["g12"], ui, start=True, stop=False)
                rg = nrepool.tile([64, TILE], bf16, tag="nr")
                eg = nrepool.tile([64, TILE], bf16, tag="ne")
                nc.scalar.activation(rg[:], zg[:], AF.Relu,
                                     bias=w["ngb1"], scale=-1.0)
                nc.scalar.activation(eg[:], rg[:], AF.Exp, scale=-1.0)
                nc.tensor.matmul(o2[:], w["g2"], rg[:], start=False,
                                 stop=False)
                nc.tensor.matmul(o2[:], w["g2"], eg[:], start=False,
                                 stop=True)
                vf = nrepool.tile([64, TILE], bf16, tag="nvf")
                mf = nrepool.tile([64, TILE], bf16, tag="nmf")
                nc.vector.tensor_scalar(vf[:], o2[:], w["gbias"], 0.0,
                                        ALU.add, ALU.max)
                nc.vector.tensor_scalar(mf[:], o2[:], w["gbias"], 0.0,
                                        ALU.add, ALU.min)
                ef = nrepool.tile([64, TILE], bf16, tag="nef")
                nc.scalar.activation(ef[:], mf[:], AF.Exp, scale=1.0)
                stt = nc.vector.scalar_tensor_tensor(
                    out_sb[:, i * TILE:(i + 1) * TILE], ef[:], -1.0, vf[:],
                    op0=ALU.add, op1=ALU.add)
                prev_stt[i] = stt
                nc.sync.dma_start(out_d[:, i * TILE:(i + 1) * TILE],
                                  out_sb[:, i * TILE:(i + 1) * TILE])

    _prune_waits(nc)
    return nc


def _prune_waits(nc):
    """ISA structs carry at most one sync wait. Drop provably-redundant
    waits Tile emitted (same-engine self-waits on strict-FIFO engines;
    DMA-vs-DMA ordering subsumed by compute waits; drain-tail waits)."""
    n1 = n2 = 0
    for b in nc.m.functions[0].blocks:
        for i in b.instructions:
            si = i.sync_info
            if si is None or not si.on_wait or len(si.on_wait) < 2:
                continue
            nm = type(i).__name__
            waits = list(si.on_wait)
            if nm == "InstDrain":
                dma_w = [x for x in waits if x.ant_name.startswith("DMAHW")]
                if dma_w:
                    si.on_wait = dma_w[-1:]
                else:
                    si.on_wait = waits[-1:]
                continue
            if nm == "InstDMACopy":
                if any(not x.ant_name.startswith("DMAHW") and
                       not x.ant_name.startswith("DMASW") for x in waits):
                    kept = [x for x in waits
                            if not (x.ant_name.startswith("DMAHW") or
                                    x.ant_name.startswith("DMASW"))]
                    n2 += len(waits) - len(kept)
                    waits = kept
            else:
                own = str(i.engine).split(".")[-1]
                kept = [x for x in waits
                        if x.ant_name.rsplit("_", 1)[0] != own]
                if len(kept) < len(waits):
                    n1 += len(waits) - len(kept)
                    waits = kept
            si.on_wait = waits
    return n1, n2


# --------------------------------------------------------------------------
# entry points
# --------------------------------------------------------------------------

def _prepare(x, pos, edge_index, f_w1, f_b1, f_w2, f_b2,
             g_w1, g_b1, g_w2, g_b2):
    x = np.asarray(x, F32)
    pos = np.asarray(pos, F32)
    src = np.asarray(edge_index[0]).astype(np.int64)
    dst = np.asarray(edge_index[1]).astype(np.int64)
    cores = _core_layouts(edge_index)
    tiles, S, ncw = _tile_plan(cores)
    S_pad = ((S + SUP - 1) // SUP) * SUP
    packs = []
    for c, core in enumerate(cores):
        feats, xnode = _pack_core(core, tiles, S_pad, ncw, x, pos, src, dst)
        xnode[:, :NCN] = x[core["order"] + c * NCN].T
        packs.append((feats, xnode))
    w = _weights(np.asarray(f_w1, F32), np.asarray(f_b1, F32),
                 np.asarray(f_w2, F32), np.asarray(f_b2, F32),
                 np.asarray(g_w1, F32), np.asarray(g_b1, F32),
                 np.asarray(g_w2, F32), np.asarray(g_b2, F32))
    return cores, tiles, S_pad, ncw, packs, w


def _finalize(results, cores, x, g_w1, g_b1, g_w2, g_b2):
    """results: list of [64, ncw] per core -> full [N, 64] output."""
    out = np.empty((N, 64), dtype=F32)
    for c, core in enumerate(cores):
        out[core["order"] + c * NCN] = np.asarray(
            results[c], F32)[:, :NCN].T
    empties = np.concatenate([c["empty"] for c in cores])
    if empties.size:
        def celu(v):
            return np.maximum(v, 0) + np.minimum(0, np.expm1(np.minimum(v, 0)))
        u_in = np.concatenate(
            [np.zeros((empties.size, 64), F32), x[empties]], axis=1)
        u = celu(u_in @ g_w1 + g_b1)
        out[empties] = celu(u @ g_w2 + g_b2).astype(F32)
    return out


def kernel(x, pos, edge_index, f_w1, f_b1, f_w2, f_b2,
           g_w1, g_b1, g_w2, g_b2, _debug_numpy=False, _trace=False):
    x = np.asarray(x, F32)
    pos = np.asarray(pos, F32)
    cores, tiles, S_pad, ncw, packs, w = _prepare(
        x, pos, edge_index, f_w1, f_b1, f_w2, f_b2, g_w1, g_b1, g_w2, g_b2)

    if _debug_numpy:
        results = [_numpy_device(f, xn, w, tiles, ncw) for (f, xn) in packs]
        return _finalize(results, cores, x, np.asarray(g_w1, F32),
                         np.asarray(g_b1, F32), np.asarray(g_w2, F32),
                         np.asarray(g_b2, F32))

    _import_concourse()
    run_kwargs = {}
    if _trace:
        _install_ntff_shim()
        import concourse.bass_utils as _bu
        _bu.upload_artifacts = lambda tmpdir: f"file://{tmpdir}"
        import tempfile
        trace_dir = tempfile.mkdtemp(prefix="bass_trace_")
        run_kwargs = dict(tmpdir=trace_dir)
        kernel._last_trace_dir = trace_dir
    from concourse.bass_utils import run_bass_kernel_spmd

    import ml_dtypes
    bf = ml_dtypes.bfloat16
    nc = _build_nc(tiles, S_pad, ncw)
    in_maps = [{"feats": feats.astype(bf), "xnode": xnode.astype(bf),
                "wpack": w["wpack"], "bpack": w["bpack"]}
               for (feats, xnode) in packs]
    res = run_bass_kernel_spmd(nc, in_maps, list(range(CORES)), trace=_trace,
                               **run_kwargs)
    results = [res.results[c]["out"] for c in range(CORES)]
    out = _finalize(results, cores, x, np.asarray(g_w1, F32),
                    np.asarray(g_b1, F32), np.asarray(g_w2, F32),
                    np.asarray(g_b2, F32))
    if _trace:
        kernel._last_exec_time_ns = res.exec_time_ns
        kernel._last_mean_exec_time_ns = res.mean_exec_time_ns
    return out


# revision 22
# speedup vs baseline: 1.2362x; 1.0254x over previous
"""Trainium2 Bass kernel for a GNN message-passing layer.

Reference semantics (per edge e = (src j, dst i)):
    m_in  = [x_j, pos_j - pos_i]                 # [E, 6]
    h     = celu(m_in @ f_w1 + f_b1)             # [E, 64]
    msg   = relu(h @ f_w2 + f_b2)                # [E, 64]
    aggr  = segment_max(msg, dst, N); empty -> 0 # [N, 64]
    u     = celu([aggr, x] @ g_w1 + g_b1)
    out   = celu(u @ g_w2 + g_b2)                # [N, 64]

Sharding: nodes split into 8 contiguous ranges (6250/core); each core gets the
edges whose dst is in its range, so segment-max is local.  Host does
index-only work (degree-sort, round layout, gather); device does every FLOP.

Device program (v2): celu decomposed as celu(z) = relu(-z) + exp(-relu(-z))
+ z - 1.  Per 1024-column group (2 edges stacked per column):
  zb = w9@f (PSUM), then either
    A-path: r = ACT.Relu(-zb-b1), e = ACT.Exp(-r); ms += w2@r + w2@e
    D-path: m = DVE.ts(zb+b1 min 0) (= -r), e = ACT.Exp(m); ms += (-w2)@m + w2@e
  ms also accumulates w12@f (the linear z term), then DVE tensor-max into a
  bf16 running aggregate (relu+bias deferred past the max).
The PE stream is software-pipelined depth-2 (w2-streams of group g run while
zb of g+2 and ms-init of g+1 are computed) so the tensor engine never waits
on ACT; a gap-free warmup burst un-throttles the PE HAM clock gate
(1.2 -> 2.4 GHz) at kernel start and keep-warm dummies span the node-phase
lead-in.
"""

import math
import os
import sys

import numpy as np

N = 50000
E = 1600000
CORES = 8
NCN = N // CORES            # nodes per core
TILE = 512                  # fp32 matmul moving free dim / one PSUM bank
GRP = 1024                  # group width (columns) = 2 tiles
SUP = 4096                  # feats DMA staging superblock (columns) = 4 groups
F32 = np.float32
DPAT = 3                    # every DPAT-th group takes the DVE (m) path


# --------------------------------------------------------------------------
# host-side layout (index work only)
# --------------------------------------------------------------------------

def _core_layouts(edge_index):
    """Per-core node ordering + degree-sorted CSR of local edges."""
    dst = np.asarray(edge_index[1])
    cores = []
    for c in range(CORES):
        lo, hi = c * NCN, (c + 1) * NCN
        eids = np.nonzero((dst >= lo) & (dst < hi))[0]
        ldst = (dst[eids] - lo).astype(np.int64)
        deg = np.bincount(ldst, minlength=NCN)
        order = np.argsort(-deg, kind="stable")         # node ranks
        rank = np.empty(NCN, np.int64)
        rank[order] = np.arange(NCN)
        perm = np.argsort(rank[ldst], kind="stable")
        es = eids[perm]                                  # edges sorted by rank
        deg_s = deg[order]
        row_start = np.zeros(NCN + 1, np.int64)
        np.cumsum(deg_s, out=row_start[1:])
        cores.append(dict(es=es, deg_s=deg_s, row_start=row_start,
                          order=order, empty=order[deg_s == 0] + lo))
    return cores


def _tile_plan(cores):
    """Shared (SPMD-uniform) tile plan at 512-column granularity.

    tiles: list of (pair_round t, node_block k); tile covers node ranks
    [512k, 512k+512) at rounds (2t, 2t+1).  Flat consecutive pairs of tiles
    form 1024-column groups (groups may straddle rounds; the aggregate-max
    is per-tile anyway).
    """
    rmax = max(int(c["deg_s"][0]) for c in cores)
    n_pairs = (rmax + 1) // 2
    tiles = []
    for t in range(n_pairs):
        w = max(int(np.searchsorted(-c["deg_s"], -(2 * t), side="left"))
                for c in cores)      # max over cores of #nodes with deg > 2t
        if t == 0:
            w = NCN                  # every aggr column gets initialized
        for k in range(max(1, (w + TILE - 1) // TILE)):
            tiles.append((t, k))
    if len(tiles) % 2:
        assert tiles[-1][0] > 0
        tiles.append(tiles[-1])      # dup: max is idempotent, not first-touch
    S = TILE * len(tiles)
    ncw = TILE * ((NCN + TILE - 1) // TILE)
    return tiles, S, ncw


def _pack_core(core, tiles, S, ncw, x, pos, src, dst):
    """Build one core's slot->edge assignment and gather features."""
    es, deg_s, row_start = core["es"], core["deg_s"], core["row_start"]
    ncols = len(tiles) * TILE
    nvec = np.tile(np.arange(TILE, dtype=np.int64), len(tiles))  # col in tile
    kvec = np.repeat([k for (_, k) in tiles], TILE)
    tvec = np.repeat([t for (t, _) in tiles], TILE)
    node = kvec * TILE + nvec                    # node rank targeted by column

    safe_node = np.minimum(node, NCN - 1)
    ecap = len(es) - 1
    first_edge = es[np.minimum(row_start[safe_node], ecap)]  # dup fallback
    bad = (node >= NCN) | (deg_s[safe_node] == 0)
    first_edge = np.where(bad, es[0], first_edge)

    def round_edges(r):
        has = (~bad) & (deg_s[safe_node] > r)
        idx = np.minimum(row_start[safe_node] + np.where(has, r, 0), ecap)
        return np.where(has, es[idx], first_edge)

    a_e = round_edges(2 * tvec)        # vectorized: r differs per column
    b_e = round_edges(2 * tvec + 1)

    # rows 0-17: features for the w1n (zb) stream; rows 32-49: the same
    # features again for the w12 (ms-init) stream, so each superblock is a
    # single rectangular DMA and the two matmul streams read disjoint
    # partition bands (array rows 0-31 / 32-63, concurrent row groups)
    feats = np.zeros((50, S), dtype=F32)
    for half, eids in ((0, a_e), (9, b_e)):
        s, d = src[eids], dst[eids]
        feats[half + 0:half + 3, :ncols] = x[s].T
        feats[half + 3:half + 6, :ncols] = pos[s].T
        feats[half + 6:half + 9, :ncols] = pos[d].T
    feats[32:50] = feats[0:18]

    xnode = np.zeros((3, ncw), dtype=F32)
    xnode[:, :NCN] = x[core["order"] + 0].T      # caller adds core offset
    return feats, xnode


# column layouts of the packed weight tensors (bf16 matmul operands; PE runs
# fp32 at 1/4 rate, bf16 streams 1 col/cycle with f32 PSUM accumulation).
# w12 lives at partitions 32-49 so its matmuls run in array rows 32-63,
# concurrent with the w1n (rows 0-31) matmuls.
WSLOTS = dict(w1n=(0, 18, 0, 128), w12=(32, 50, 128, 128),
              w2p=(0, 128, 256, 128), g1n=(0, 67, 384, 64),
              g12=(0, 67, 448, 64), g2=(0, 64, 512, 64))
WCOL = 576
BSLOTS = dict(nbias1=(128, 0, 1), cbias=(64, 1, 1), ngb1=(64, 2, 1),
              pgb1=(64, 3, 1), gbias=(64, 4, 1), gbm1=(64, 5, 1))
BCOL = 8


def _weights(f_w1, f_b1, f_w2, f_b2, g_w1, g_b1, g_w2, g_b2):
    w9 = np.concatenate([f_w1[0:3], f_w1[3:6], -f_w1[3:6]], axis=0)  # [9,64]
    blk = lambda m: np.block([[m, np.zeros_like(m)], [np.zeros_like(m), m]])
    cbias = (f_b1 @ f_w2 - f_w2.sum(axis=0) + f_b2).astype(F32)       # [64]
    gbias = (g_b1 @ g_w2 - g_w2.sum(axis=0) + g_b2).astype(F32)       # [64]
    w = dict(
        w1n=blk(w9).astype(F32),             # [18,128]  (zb = +z1)
        w12=blk(w9 @ f_w2).astype(F32),      # [18,128]
        w2p=blk(f_w2).astype(F32),           # [128,128]
        g1n=g_w1.astype(F32),                # [67,64]
        g12=(g_w1 @ g_w2).astype(F32),       # [67,64]
        g2=g_w2.astype(F32),                 # [64,64]
        nbias1=np.tile(-f_b1, 2).astype(F32).reshape(128, 1),
        cbias=cbias.reshape(64, 1),
        ngb1=(-g_b1).astype(F32).reshape(64, 1),
        pgb1=g_b1.astype(F32).reshape(64, 1),
        gbias=gbias.reshape(64, 1),
        gbm1=(gbias - 1.0).reshape(64, 1),
    )
    import ml_dtypes
    wpack = np.zeros((128, WCOL), dtype=ml_dtypes.bfloat16)
    for name, (p0, p1, c0, cn) in WSLOTS.items():
        wpack[p0:p1, c0:c0 + cn] = w[name]
    bpack = np.zeros((128, BCOL), dtype=F32)
    for name, (p, c0, cn) in BSLOTS.items():
        bpack[:p, c0:c0 + cn] = w[name]
    w["wpack"] = wpack
    w["bpack"] = bpack
    return w


def _bf(v):
    import ml_dtypes
    return np.asarray(v).astype(ml_dtypes.bfloat16).astype(F32)


# --------------------------------------------------------------------------
# numpy model of the device program (for validation; mimics bf16 rounding)
# --------------------------------------------------------------------------

def _numpy_device(feats, xnode, w, tiles, ncw):
    G = len(tiles) // 2
    aggr = np.zeros((128, ncw), dtype=F32)
    for g in range(G):
        f = _bf(feats[:, g * GRP:(g + 1) * GRP])
        zb = w["w1n"].T @ f                                  # +z1
        r = _bf(np.maximum(-zb + w["nbias1"], 0))
        e = _bf(np.exp(-r))
        s = _bf(r + e)
        ms = w["w12"].T @ f + w["w2p"].T @ s
        for j in (0, 1):
            t, k = tiles[2 * g + j]
            dst = aggr[:, k * TILE:(k + 1) * TILE]
            src = _bf(ms[:, j * TILE:(j + 1) * TILE])
            if t == 0:
                dst[:] = src
            else:
                np.maximum(dst, src, out=dst)
    a64 = np.maximum(aggr[0:64], aggr[64:128])
    u_in = np.empty((67, ncw), dtype=F32)
    u_in[0:64] = _bf(np.maximum(a64 + w["cbias"], 0))
    u_in[64:67] = _bf(xnode)
    out = np.empty((64, ncw), dtype=F32)
    for i in range(ncw // TILE):
        ui = u_in[:, i * TILE:(i + 1) * TILE]
        zg = w["g1n"].T @ ui
        rg = _bf(np.maximum(-zg + w["ngb1"], 0))
        y2 = _bf(np.exp(zg + w["pgb1"]))
        sg = _bf(np.minimum(y2, 1.0) + rg)
        o2 = w["g12"].T @ ui + w["g2"].T @ sg
        y = _bf(np.exp(o2 + w["gbias"]))
        vf2 = _bf(np.maximum(o2 + w["gbm1"], -1.0))
        out[:, i * TILE:(i + 1) * TILE] = _bf(np.minimum(y, 1.0) + vf2)
    return out        # [64, ncw] (bf16-rounded values)


# --------------------------------------------------------------------------
# bass program
# --------------------------------------------------------------------------

def _import_concourse():
    try:
        import concourse.bass  # noqa: F401
    except ImportError:
        sys.path.insert(0, "/opt/trn_rl_repo")


def _install_ntff_shim():
    """Provide antenv.axon_hooks (missing in this image) so that
    run_bass_kernel_spmd(trace=True) can capture NTFF profiles."""
    import contextlib
    import ctypes
    import types

    if "antenv.axon_hooks" in sys.modules:
        return
    so_path = "/opt/axon/libaxon_pjrt.so"
    if not os.path.exists(so_path):
        return
    lib = ctypes.CDLL(so_path)
    if not hasattr(lib, "axon_start_nrt_profile"):
        return
    lib.axon_start_nrt_profile.argtypes = [ctypes.POINTER(ctypes.c_int64),
                                           ctypes.c_size_t]
    lib.axon_start_nrt_profile.restype = ctypes.c_int64
    lib.axon_stop_nrt_profile.argtypes = [ctypes.c_char_p]
    lib.axon_stop_nrt_profile.restype = ctypes.c_int64

    @contextlib.contextmanager
    def _hook(output_dir, device_ids):
        import jax
        jax.devices()
        if device_ids:
            ids = (ctypes.c_int64 * len(device_ids))(*device_ids)
            rc = lib.axon_start_nrt_profile(ids, len(device_ids))
        else:
            rc = lib.axon_start_nrt_profile(None, 0)
        if rc != 0:
            raise RuntimeError(f"axon_start_nrt_profile rc={rc}")
        try:
            yield
        finally:
            n = lib.axon_stop_nrt_profile(str(output_dir).encode())
            print(f"ntff profile: {n} file(s) -> {output_dir}",
                  file=sys.stderr)

    mod = types.ModuleType("antenv.axon_hooks")
    mod.get_axon_ntff_profile_hook = lambda: _hook
    mod.set_axon_ntff_profile_hook = lambda h: None
    sys.modules["antenv.axon_hooks"] = mod


def _dep(from_inst, to_inst, reason):
    from concourse.tile import add_dep_helper
    a = getattr(from_inst, "ins", from_inst)
    b = getattr(to_inst, "ins", to_inst)
    add_dep_helper(a, b, reason=reason)


def _build_nc(tiles, S, ncw):
    _import_concourse()
    import concourse.bass as bass
    import concourse.tile as tile
    import concourse.tile_sem_assignment as _tsa
    from concourse import mybir

    # One DMAHW bookkeeping lane: HWDGE transfers share a FIFO proc, so
    # DMA-vs-DMA ordering (slot WAW) needs no extra sync wait.
    _tsa.NUM_HWDGE_SEMS = 1

    f32 = mybir.dt.float32
    bf16 = mybir.dt.bfloat16
    AF = mybir.ActivationFunctionType
    ALU = mybir.AluOpType
    nc = bass.Bass()

    G = len(tiles) // 2
    S_pad = ((S + SUP - 1) // SUP) * SUP
    n_sup = S_pad // SUP
    n_nt = ncw // TILE                       # node tiles

    feats_d = nc.dram_tensor("feats", [50, S_pad], bf16, kind="ExternalInput")
    xnode_d = nc.dram_tensor("xnode", [3, ncw], bf16, kind="ExternalInput")
    wpack_d = nc.dram_tensor("wpack", [128, WCOL], bf16, kind="ExternalInput")
    bpack_d = nc.dram_tensor("bpack", [128, BCOL], f32, kind="ExternalInput")
    out_d = nc.dram_tensor("out", [64, ncw], bf16, kind="ExternalOutput")

    # node-phase lead-in chunks (4 tiles each) and the edge-group after which
    # each chunk's aggr columns are final (chunk 0 = blocks 0-3 is last)
    n_ck = (n_nt + 3) // 4
    ck_last = []
    for c in range(n_ck):
        blocks = set(range(4 * c, min(4 * c + 4, n_nt)))
        last = 0
        for j, (t, k) in enumerate(tiles):
            if k in blocks:
                last = j // 2
        ck_last.append(last)

    with tile.TileContext(nc) as tc:
        with (
            tc.tile_pool(name="const", bufs=1) as cpool,
            tc.tile_pool(name="aggr", bufs=1) as apool,
            tc.tile_pool(name="feats", bufs=2) as fpool,
            tc.tile_pool(name="rm", bufs=3) as rmpool,
            tc.tile_pool(name="et", bufs=2) as etpool,
            tc.tile_pool(name="st", bufs=2) as stpool,
            tc.tile_pool(name="gwork", bufs=1) as gpool,
            tc.tile_pool(name="nrg", bufs=3) as nrgpool,
            tc.tile_pool(name="ny2", bufs=2) as ny2pool,
            tc.tile_pool(name="ny", bufs=3) as nypool,
            tc.tile_pool(name="nsg", bufs=2) as nsgpool,
            tc.tile_pool(name="nvf", bufs=2) as nvfpool,
            tc.tile_pool(name="psum_z", bufs=2, space="PSUM") as pz,
            tc.tile_pool(name="psum_m", bufs=2, space="PSUM") as pm,
        ):
            wsb = cpool.tile([128, WCOL], bf16, name="wsb")
            wdma = nc.sync.dma_start(wsb[:], wpack_d[:])
            bsb = cpool.tile([128, BCOL], f32, name="bsb")
            bdma = nc.sync.dma_start(bsb[:], bpack_d[:])
            w = {name: wsb[p0:p1, c0:c0 + cn]
                 for name, (p0, p1, c0, cn) in WSLOTS.items()}
            w.update({name: bsb[0:p, c0:c0 + cn]
                      for name, (p, c0, cn) in BSLOTS.items()})
            # ACT/DVE-side absorbers: observe the bias DMA once.
            tabs = cpool.tile([1, 8], f32, name="tabs")
            ta0 = nc.scalar.activation(tabs[0:1, 0:1], bsb[0:1, 0:1], AF.Copy)
            _dep(ta0, bdma, "ACT observes bias DMA")
            vscr = cpool.tile([1, 8], f32, name="vscr")
            tv0 = nc.vector.tensor_copy(vscr[0:1, 0:1], bsb[0:1, 0:1])
            _dep(tv0, bdma, "DVE observes bias DMA")

            aggr = apool.tile([128, ncw], bf16)
            u_in = gpool.tile([67, ncw], bf16, tag="u_in")
            ah = gpool.tile([64, ncw], bf16, tag="ah")
            out_sb = gpool.tile([64, ncw], bf16, tag="out_sb")

            # ---- feats superblock staging: rows 0-17 feed the w1n (zb)
            # stream in array rows 0-31; a second copy at partitions 32-49
            # feeds the w12 (ms-init) stream in array rows 32-63 so both
            # matmuls run concurrently in different row groups.
            sup_tiles = [None] * n_sup
            sup_dmas = [None] * n_sup
            def stage_sup(i):
                st_ = fpool.tile([50, SUP], bf16, tag="feats_sup")
                d = nc.sync.dma_start(st_[:],
                                      feats_d[:, i * SUP:(i + 1) * SUP])
                sup_tiles[i] = st_
                sup_dmas[i] = d
            for i in range(min(2, n_sup)):
                stage_sup(i)

            def fcols(g, band):
                c0 = g * GRP
                st_ = sup_tiles[c0 // SUP]
                fo = c0 % SUP
                if band == 0:
                    return st_[0:18, fo:fo + GRP]
                return st_[32:50, fo:fo + GRP]

            def emit_zb(g, off):
                """one 512-col zb matmul (array rows 0-31)."""
                fa = fcols(g, 0)
                if off == 0:
                    zbt = pz.tile([128, GRP], f32, tag="zb")
                    emit_zb.cur = zbt
                zbt = emit_zb.cur
                mm = nc.tensor.matmul(zbt[:, off:off + TILE], w["w1n"],
                                      fa[:, off:off + TILE], start=True,
                                      stop=True)
                return zbt, mm

            def emit_ms(g, off):
                """one 512-col ms-init matmul (array rows 32-63)."""
                fa = fcols(g, 1)
                if off == 0:
                    mst = pm.tile([128, GRP], f32, tag="ms")
                    emit_ms.cur = mst
                mst = emit_ms.cur
                mm = nc.tensor.matmul(mst[:, off:off + TILE], w["w12"],
                                      fa[:, off:off + TILE], start=True,
                                      stop=False)
                return mst, mm

            def emit_zbms(gz, gm, after=None):
                """interleaved concurrent pairs: zb(gz) rows 0-31 overlaps
                ms-init(gm) rows 32-63; `after` pins PE order behind the
                w2 twin so its sup-DMA wait covers these reads."""
                zbt = mst = None
                first = True
                for off in (0, TILE):
                    if gz is not None:
                        zbt, mm = emit_zb(gz, off)
                        if first and after is not None:
                            _dep(mm, after, "order after w2 twin")
                            first = False
                    if gm is not None:
                        mst, mm = emit_ms(gm, off)
                        if first and after is not None:
                            _dep(mm, after, "order after w2 twin")
                            first = False
                return zbt, mst

            def emit_re(g, zbt, prev_e):
                """r = relu(-zb-b1) [ACT], e = exp(-r) [ACT].

                r is chained after the previous group's e: that e already
                waits on the DVE s-op releasing the rm slot r reuses (rm
                bufs=3 vs et bufs=2 alignment), so r keeps only its PE wait.
                """
                rm = rmpool.tile([128, GRP], bf16, tag="rm")
                et = etpool.tile([128, GRP], bf16, tag="et")
                ri = nc.scalar.activation(rm[:], zbt[:], AF.Relu,
                                          bias=w["nbias1"], scale=-1.0)
                if prev_e is not None:
                    _dep(ri, prev_e, "rm slot WAR covered by prev e wait")
                ei = nc.scalar.activation(et[:], rm[:], AF.Exp, scale=-1.0)
                return rm, et, ei

            def emit_s(g, rm, et, prev_agg):
                """s = r + e [DVE, bf16 2x] into a PE-only tile; chained
                after the latest aggmax so its st-slot PE-WAR is covered."""
                st_ = stpool.tile([128, GRP], bf16, tag="st")
                si = nc.vector.tensor_add(st_[:], rm[:], et[:])
                if prev_agg is not None:
                    _dep(si, prev_agg, "st slot WAR covered by aggmax wait")
                return st_

            def emit_w2(g, mst, st_):
                mm = []
                for off in (0, TILE):
                    mm.append(nc.tensor.matmul(
                        mst[:, off:off + TILE], w["w2p"],
                        st_[:, off:off + TILE], start=False, stop=True))
                return mm

            def emit_aggmax(g, mst):
                outs = []
                for j in (0, 1):
                    t, k = tiles[2 * g + j]
                    dst = aggr[:, k * TILE:(k + 1) * TILE]
                    src = mst[:, j * TILE:(j + 1) * TILE]
                    if t == 0:
                        outs.append(nc.vector.tensor_copy(dst, src))
                    else:
                        outs.append(nc.vector.tensor_max(dst, dst, src))
                return outs

            # ---- node-phase lead-in (per 4-tile chunk): move odd-round half
            # down, fold max, relu+cbias into u_in; emitted as soon as the
            # chunk's aggr columns are final so it hides under the edge phase
            def emit_chunk(c):
                c0 = 4 * c * TILE
                cw = min(ncw - c0, 4 * TILE)
                d = nc.sync.dma_start(ah[:, c0:c0 + cw],
                                      aggr[64:128, c0:c0 + cw])
                tvc = nc.vector.tensor_copy(vscr[0:1, 1:2], bsb[0:1, 0:1])
                _dep(tvc, d, "DVE absorbs fold DMA dep")
                fo = nc.vector.tensor_max(ah[:, c0:c0 + cw],
                                          aggr[0:64, c0:c0 + cw],
                                          ah[:, c0:c0 + cw])
                _dep(fo, tvc, "order after absorber")
                ur = nc.scalar.activation(u_in[0:64, c0:c0 + cw],
                                          ah[:, c0:c0 + cw], AF.Relu,
                                          bias=w["cbias"], scale=1.0)
                return ur

            # =========== edge phase ===========
            zb_t = {}
            ms_t = {}
            rm_t = {}
            et_t = {}
            st_t = {}
            zb_t[0], ms_t[0] = emit_zbms(0, 0)
            zb_t[1], _ = emit_zbms(1, None)
            rm_t[0], et_t[0], prev_e = emit_re(0, zb_t[0], None)
            st_t[0] = emit_s(0, rm_t[0], et_t[0], None)
            prev_agg = None

            chunks_done = set()
            chunk_insts = {}
            for g in range(G):
                mm_e = emit_w2(g, ms_t[g], st_t.pop(g))
                rm_t.pop(g)
                et_t.pop(g)
                # if groups g+1/g+2 (read next on PE) start a new superblock,
                # absorb that DMA's wait on the wait-free w2 twin
                c_nxt = (g + 2) * GRP
                if g + 2 < G and c_nxt % SUP == 0 \
                        and sup_dmas[c_nxt // SUP] is not None:
                    _dep(mm_e[1], sup_dmas[c_nxt // SUP],
                         "sup prefetch via w2 twin")
                prev_agg = emit_aggmax(g, ms_t.pop(g))[-1]
                # prefetch the superblock that group g+3 will read
                c3 = (g + 3) * GRP
                if g + 3 < G and c3 % SUP == 0 and c3 // SUP < n_sup \
                        and sup_tiles[c3 // SUP] is None:
                    stage_sup(c3 // SUP)
                gz = g + 2 if g + 2 < G else None
                gm = g + 1 if g + 1 < G else None
                if gz is not None or gm is not None:
                    zbt, mst = emit_zbms(gz, gm, after=mm_e[1])
                    if gz is not None:
                        zb_t[gz] = zbt
                    if gm is not None:
                        ms_t[gm] = mst
                if g + 1 < G:
                    rm_t[g + 1], et_t[g + 1], prev_e = emit_re(
                        g + 1, zb_t[g + 1], prev_e)
                    st_t[g + 1] = emit_s(g + 1, rm_t[g + 1], et_t[g + 1],
                                         prev_agg)
                # early node-phase chunks once their blocks are final
                for c in range(1, n_ck):
                    if c not in chunks_done and ck_last[c] == g:
                        chunks_done.add(c)
                        chunk_insts[c] = emit_chunk(c)

            xdma = nc.sync.dma_start(u_in[64:67, :], xnode_d[:])
            chunk_insts[0] = emit_chunk(0)
            for c in range(1, n_ck):
                if c not in chunks_done:
                    chunk_insts[c] = emit_chunk(c)

            # absorber matmuls: observe the chunk-0 relu (ACT) and the xnode
            # DMA so the first node matmuls keep a single sem wait
            kwt1 = pz.tile([64, TILE], f32, tag="zb", name="kwt1")
            kw1 = nc.tensor.matmul(kwt1[:], wsb[0:64, 0:64],
                                   wsb[0:64, 0:TILE], start=True, stop=True)
            _dep(kw1, chunk_insts[0], "observe chunk-0 relu")
            kwt2 = pz.tile([64, TILE], f32, tag="zb", name="kwt2")
            kw2 = nc.tensor.matmul(kwt2[:], wsb[0:64, 0:64],
                                   wsb[0:64, 0:TILE], start=True, stop=True)
            _dep(kw2, xdma, "absorb xnode DMA wait")

            # =========== node phase ===========
            # per tile: zg = g1n@u; rg = relu(-zg-gb1); y2 = exp(zg+gb1);
            # sg = min(y2,1)+rg; o2 = g12@u + g2@sg; y = exp(o2+gbias);
            # vf2 = max(o2+gbias-1, -1); out = min(y,1)+vf2.
            # Explicit deps keep every instruction at <=1 sync wait.
            zg_t = {}
            o2_t = {}

            def emit_nmm1(i, prev_g2sg):
                ui = u_in[:, i * TILE:(i + 1) * TILE]
                zg = pz.tile([64, TILE], f32, tag="zb")
                nc.tensor.matmul(zg[:], w["g1n"], ui, start=True, stop=True)
                o2 = pm.tile([64, TILE], f32, tag="ms")
                o2i = nc.tensor.matmul(o2[:], w["g12"], ui, start=True,
                                       stop=False)
                if prev_g2sg is not None:
                    _dep(o2i, prev_g2sg, "pm slot DVE-WAR covered by g2sg")
                return zg, o2

            def emit_nact(i, prev_y2):
                rg = nrgpool.tile([64, TILE], bf16, tag="nrg")
                y2 = ny2pool.tile([64, TILE], bf16, tag="ny2")
                rgi = nc.scalar.activation(rg[:], zg_t[i][:], AF.Relu,
                                           bias=w["ngb1"], scale=-1.0)
                if prev_y2 is not None:
                    _dep(rgi, prev_y2, "nrg slot WAR covered by prev y2")
                y2i = nc.scalar.activation(y2[:], zg_t[i][:], AF.Exp,
                                           bias=w["pgb1"], scale=1.0)
                _dep(y2i, rgi, "share zg PE wait")
                return rg, y2, y2i

            def emit_nsg(i, rg, y2, prev_vf2):
                sg = nsgpool.tile([64, TILE], bf16, tag="nsg")
                sgi = nc.vector.scalar_tensor_tensor(
                    sg[:], y2[:], 1.0, rg[:], op0=ALU.min, op1=ALU.add)
                if prev_vf2 is not None:
                    _dep(sgi, prev_vf2, "nsg slot PE-WAR covered by vf2")
                return sg

            zg_t[0], o2_t[0] = emit_nmm1(0, None)
            rg_c, y2_c, y2i_c = emit_nact(0, None)
            sg_t = {0: emit_nsg(0, rg_c, y2_c, None)}
            prev_y2i = y2i_c
            prev_vf2 = None

            for i in range(n_nt):
                o2 = o2_t.pop(i)
                mm_o2 = nc.tensor.matmul(o2[:], w["g2"], sg_t.pop(i)[:],
                                         start=False, stop=True)
                if i + 1 < n_nt:
                    zg_t[i + 1], o2_t[i + 1] = emit_nmm1(i + 1, mm_o2)
                    rg_c, y2_c, y2i_c = emit_nact(i + 1, prev_y2i)
                    prev_y2i = y2i_c
                y = nypool.tile([64, TILE], bf16, tag="ny")
                yi = nc.scalar.activation(y[:], o2[:], AF.Exp,
                                          bias=w["gbias"], scale=1.0)
                _dep(yi, prev_y2i, "ny slot DVE-WAR covered by y2 wait")
                vf2 = nvfpool.tile([64, TILE], bf16, tag="nvf")
                vf2i = nc.vector.tensor_scalar(vf2[:], o2[:], w["gbm1"], -1.0,
                                               ALU.add, ALU.max)
                nc.vector.scalar_tensor_tensor(
                    out_sb[:, i * TILE:(i + 1) * TILE], y[:], 1.0, vf2[:],
                    op0=ALU.min, op1=ALU.add)
                if i + 1 < n_nt:
                    sg_t[i + 1] = emit_nsg(i + 1, rg_c, y2_c, vf2i)
                nc.sync.dma_start(out_d[:, i * TILE:(i + 1) * TILE],
                                  out_sb[:, i * TILE:(i + 1) * TILE])

    return nc


def _set_waits(inst, kept):
    """Replace an instruction's sync waits.  inst.sync_info returns a copy,
    so rebuild fresh SyncWait/SyncInfo objects and assign them back to the
    instruction."""
    import bass_rust
    news = [bass_rust.SyncWait(sync_type=x.sync_type, id=x.id,
                               wait_mode=x.wait_mode, ant_name=x.ant_name,
                               wait_value=x.wait_value, wait_reg=x.wait_reg)
            for x in kept]
    si = inst.sync_info
    ups = [bass_rust.SyncUpdate(sync_type=u.sync_type, id=u.id,
                                ant_name=u.ant_name, update_value=u.update_value)
           if False else u for u in list(si.on_update)]
    inst.sync_info = bass_rust.SyncInfo(on_wait=news, on_update=ups)


def _prune_waits(nc):
    """ISA structs carry at most one sync wait. Drop provably-redundant
    waits Tile emitted (same-engine self-waits on strict-FIFO engines;
    DMA-vs-DMA ordering subsumed by compute waits; drain-tail waits)."""
    n1 = n2 = 0
    for b in nc.m.functions[0].blocks:
        for i in b.instructions:
            si = i.sync_info
            if si is None or not si.on_wait or len(si.on_wait) < 2:
                continue
            nm = type(i).__name__
            waits = list(si.on_wait)
            if nm == "InstDrain":
                dma_w = [x for x in waits if x.ant_name.startswith("DMAHW")]
                _set_waits(i, dma_w[-1:] if dma_w else waits[-1:])
                continue
            if nm == "InstMatmult":
                act_w = [x for x in waits
                         if x.ant_name.startswith("Activation")]
                dma_w = [x for x in waits if x.ant_name.startswith("DMAHW")]
                if act_w and dma_w and len(act_w) + len(dma_w) == len(waits):
                    # sup-boundary zb matmul: its ACT slot-WAR (r read of the
                    # zb slot two groups back) is transitively enforced by the
                    # preceding w2 matmul's DVE wait (w2 <- s <- e <- r, ACT
                    # strict FIFO), so only the feats-DMA RAW must remain.
                    n1 += len(act_w)
                    _set_waits(i, dma_w)
                    continue
            if nm == "InstDMACopy":
                kept = [x for x in waits
                        if not (x.ant_name.startswith("DMAHW") or
                                x.ant_name.startswith("DMASW"))]
                if kept and len(kept) < len(waits):
                    n2 += len(waits) - len(kept)
                    _set_waits(i, kept)
                continue
            own = str(i.engine).split(".")[-1]
            kept = [x for x in waits
                    if x.ant_name.rsplit("_", 1)[0] != own]
            if len(kept) < len(waits):
                n1 += len(waits) - len(kept)
                _set_waits(i, kept)
    return n1, n2


# --------------------------------------------------------------------------
# entry points
# --------------------------------------------------------------------------

def _prepare(x, pos, edge_index, f_w1, f_b1, f_w2, f_b2,
             g_w1, g_b1, g_w2, g_b2):
    x = np.asarray(x, F32)
    pos = np.asarray(pos, F32)
    src = np.asarray(edge_index[0]).astype(np.int64)
    dst = np.asarray(edge_index[1]).astype(np.int64)
    cores = _core_layouts(edge_index)
    tiles, S, ncw = _tile_plan(cores)
    S_pad = ((S + SUP - 1) // SUP) * SUP
    packs = []
    for c, core in enumerate(cores):
        feats, xnode = _pack_core(core, tiles, S_pad, ncw, x, pos, src, dst)
        xnode[:, :NCN] = x[core["order"] + c * NCN].T
        packs.append((feats, xnode))
    w = _weights(np.asarray(f_w1, F32), np.asarray(f_b1, F32),
                 np.asarray(f_w2, F32), np.asarray(f_b2, F32),
                 np.asarray(g_w1, F32), np.asarray(g_b1, F32),
                 np.asarray(g_w2, F32), np.asarray(g_b2, F32))
    return cores, tiles, S_pad, ncw, packs, w


def _finalize(results, cores, x, g_w1, g_b1, g_w2, g_b2):
    """results: list of [64, ncw] per core -> full [N, 64] output."""
    out = np.empty((N, 64), dtype=F32)
    for c, core in enumerate(cores):
        out[core["order"] + c * NCN] = np.asarray(
            results[c], F32)[:, :NCN].T
    empties = np.concatenate([c["empty"] for c in cores])
    if empties.size:
        def celu(v):
            return np.maximum(v, 0) + np.minimum(0, np.expm1(np.minimum(v, 0)))
        u_in = np.concatenate(
            [np.zeros((empties.size, 64), F32), x[empties]], axis=1)
        u = celu(u_in @ g_w1 + g_b1)
        out[empties] = celu(u @ g_w2 + g_b2).astype(F32)
    return out


def kernel(x, pos, edge_index, f_w1, f_b1, f_w2, f_b2,
           g_w1, g_b1, g_w2, g_b2, _debug_numpy=False, _trace=False):
    x = np.asarray(x, F32)
    pos = np.asarray(pos, F32)
    cores, tiles, S_pad, ncw, packs, w = _prepare(
        x, pos, edge_index, f_w1, f_b1, f_w2, f_b2, g_w1, g_b1, g_w2, g_b2)

    if _debug_numpy:
        results = [_numpy_device(f, xn, w, tiles, ncw) for (f, xn) in packs]
        return _finalize(results, cores, x, np.asarray(g_w1, F32),
                         np.asarray(g_b1, F32), np.asarray(g_w2, F32),
                         np.asarray(g_b2, F32))

    _import_concourse()
    run_kwargs = {}
    if _trace:
        _install_ntff_shim()
        import concourse.bass_utils as _bu
        _bu.upload_artifacts = lambda tmpdir: f"file://{tmpdir}"
        import tempfile
        trace_dir = tempfile.mkdtemp(prefix="bass_trace_")
        run_kwargs = dict(tmpdir=trace_dir)
        kernel._last_trace_dir = trace_dir
    from concourse.bass_utils import run_bass_kernel_spmd

    import ml_dtypes
    bf = ml_dtypes.bfloat16
    nc = _build_nc(tiles, S_pad, ncw)
    _prune_waits(nc)
    in_maps = [{"feats": feats.astype(bf), "xnode": xnode.astype(bf),
                "wpack": w["wpack"], "bpack": w["bpack"]}
               for (feats, xnode) in packs]
    res = run_bass_kernel_spmd(nc, in_maps, list(range(CORES)), trace=_trace,
                               **run_kwargs)
    results = [res.results[c]["out"] for c in range(CORES)]
    out = _finalize(results, cores, x, np.asarray(g_w1, F32),
                    np.asarray(g_b1, F32), np.asarray(g_w2, F32),
                    np.asarray(g_b2, F32))
    if _trace:
        kernel._last_exec_time_ns = res.exec_time_ns
        kernel._last_mean_exec_time_ns = res.mean_exec_time_ns
    return out


# revision 24
# speedup vs baseline: 1.5620x; 1.2636x over previous
"""Trainium2 Bass kernel for a GNN message-passing layer.

Reference semantics (per edge e = (src j, dst i)):
    m_in  = [x_j, pos_j - pos_i]                 # [E, 6]
    h     = celu(m_in @ f_w1 + f_b1)             # [E, 64]
    msg   = relu(h @ f_w2 + f_b2)                # [E, 64]
    aggr  = segment_max(msg, dst, N); empty -> 0 # [N, 64]
    u     = celu([aggr, x] @ g_w1 + g_b1)
    out   = celu(u @ g_w2 + g_b2)                # [N, 64]

Sharding: nodes split into 8 contiguous ranges (6250/core); each core gets the
edges whose dst is in its range, so segment-max is local.  Host does
index-only work (degree-sort, round layout, gather); device does every FLOP.

Device program (v2): celu decomposed as celu(z) = relu(-z) + exp(-relu(-z))
+ z - 1.  Per 1024-column group (2 edges stacked per column):
  zb = w9@f (PSUM), then either
    A-path: r = ACT.Relu(-zb-b1), e = ACT.Exp(-r); ms += w2@r + w2@e
    D-path: m = DVE.ts(zb+b1 min 0) (= -r), e = ACT.Exp(m); ms += (-w2)@m + w2@e
  ms also accumulates w12@f (the linear z term), then DVE tensor-max into a
  bf16 running aggregate (relu+bias deferred past the max).
The PE stream is software-pipelined depth-2 (w2-streams of group g run while
zb of g+2 and ms-init of g+1 are computed) so the tensor engine never waits
on ACT; a gap-free warmup burst un-throttles the PE HAM clock gate
(1.2 -> 2.4 GHz) at kernel start and keep-warm dummies span the node-phase
lead-in.
"""

import math
import os
import sys

import numpy as np

N = 50000
E = 1600000
CORES = 8
NCN = N // CORES            # nodes per core
TILE = 512                  # fp32 matmul moving free dim / one PSUM bank
GRP = 1024                  # group width (columns) = 2 tiles
SUP = 4096                  # feats DMA staging superblock (columns) = 4 groups
F32 = np.float32
DPAT = 3                    # every DPAT-th group takes the DVE (m) path


# --------------------------------------------------------------------------
# host-side layout (index work only)
# --------------------------------------------------------------------------

def _core_layouts(edge_index):
    """Per-core node ordering + degree-sorted CSR of local edges."""
    dst = np.asarray(edge_index[1])
    cores = []
    for c in range(CORES):
        lo, hi = c * NCN, (c + 1) * NCN
        eids = np.nonzero((dst >= lo) & (dst < hi))[0]
        ldst = (dst[eids] - lo).astype(np.int64)
        deg = np.bincount(ldst, minlength=NCN)
        order = np.argsort(-deg, kind="stable")         # node ranks
        rank = np.empty(NCN, np.int64)
        rank[order] = np.arange(NCN)
        perm = np.argsort(rank[ldst], kind="stable")
        es = eids[perm]                                  # edges sorted by rank
        deg_s = deg[order]
        row_start = np.zeros(NCN + 1, np.int64)
        np.cumsum(deg_s, out=row_start[1:])
        cores.append(dict(es=es, deg_s=deg_s, row_start=row_start,
                          order=order, empty=order[deg_s == 0] + lo))
    return cores


def _tile_plan(cores):
    """Shared (SPMD-uniform) tile plan at 512-column granularity.

    tiles: list of (pair_round t, node_block k); tile covers node ranks
    [512k, 512k+512) at rounds (2t, 2t+1).  Flat consecutive pairs of tiles
    form 1024-column groups (groups may straddle rounds; the aggregate-max
    is per-tile anyway).
    """
    rmax = max(int(c["deg_s"][0]) for c in cores)
    n_pairs = (rmax + 1) // 2
    tiles = []
    for t in range(n_pairs):
        w = max(int(np.searchsorted(-c["deg_s"], -(2 * t), side="left"))
                for c in cores)      # max over cores of #nodes with deg > 2t
        if t == 0:
            w = NCN                  # every aggr column gets initialized
        for k in range(max(1, (w + TILE - 1) // TILE)):
            tiles.append((t, k))
    if len(tiles) % 2:
        assert tiles[-1][0] > 0
        tiles.append(tiles[-1])      # dup: max is idempotent, not first-touch
    S = TILE * len(tiles)
    ncw = TILE * ((NCN + TILE - 1) // TILE)
    return tiles, S, ncw


def _pack_core(core, tiles, S, ncw, x, pos, src, dst):
    """Build one core's slot->edge assignment and gather features."""
    es, deg_s, row_start = core["es"], core["deg_s"], core["row_start"]
    ncols = len(tiles) * TILE
    nvec = np.tile(np.arange(TILE, dtype=np.int64), len(tiles))  # col in tile
    kvec = np.repeat([k for (_, k) in tiles], TILE)
    tvec = np.repeat([t for (t, _) in tiles], TILE)
    node = kvec * TILE + nvec                    # node rank targeted by column

    safe_node = np.minimum(node, NCN - 1)
    ecap = len(es) - 1
    first_edge = es[np.minimum(row_start[safe_node], ecap)]  # dup fallback
    bad = (node >= NCN) | (deg_s[safe_node] == 0)
    first_edge = np.where(bad, es[0], first_edge)

    def round_edges(r):
        has = (~bad) & (deg_s[safe_node] > r)
        idx = np.minimum(row_start[safe_node] + np.where(has, r, 0), ecap)
        return np.where(has, es[idx], first_edge)

    a_e = round_edges(2 * tvec)        # vectorized: r differs per column
    b_e = round_edges(2 * tvec + 1)

    # rows 0-17: features for the w1n (zb) stream; rows 32-49: the same
    # features again for the w12 (ms-init) stream, so each superblock is a
    # single rectangular DMA and the two matmul streams read disjoint
    # partition bands (array rows 0-31 / 32-63, concurrent row groups)
    feats = np.zeros((50, S), dtype=F32)
    for half, eids in ((0, a_e), (9, b_e)):
        s, d = src[eids], dst[eids]
        feats[half + 0:half + 3, :ncols] = x[s].T
        feats[half + 3:half + 6, :ncols] = pos[s].T
        feats[half + 6:half + 9, :ncols] = pos[d].T
    feats[32:50] = feats[0:18]

    xnode = np.zeros((3, ncw), dtype=F32)
    xnode[:, :NCN] = x[core["order"] + 0].T      # caller adds core offset
    return feats, xnode


# column layouts of the packed weight tensors (bf16 matmul operands; PE runs
# fp32 at 1/4 rate, bf16 streams 1 col/cycle with f32 PSUM accumulation).
# w12 lives at partitions 32-49 so its matmuls run in array rows 32-63,
# concurrent with the w1n (rows 0-31) matmuls.
WSLOTS = dict(w1n=(0, 18, 0, 128), w12=(32, 50, 128, 128),
              w2p=(0, 128, 256, 128), g1n=(0, 67, 384, 64),
              g12=(0, 67, 448, 64), g2=(0, 64, 512, 64))
WCOL = 576
BSLOTS = dict(nbias1=(128, 0, 1), cbias=(64, 1, 1), ngb1=(64, 2, 1),
              pgb1=(64, 3, 1), gbias=(64, 4, 1), gbm1=(64, 5, 1))
BCOL = 8


def _weights(f_w1, f_b1, f_w2, f_b2, g_w1, g_b1, g_w2, g_b2):
    w9 = np.concatenate([f_w1[0:3], f_w1[3:6], -f_w1[3:6]], axis=0)  # [9,64]
    blk = lambda m: np.block([[m, np.zeros_like(m)], [np.zeros_like(m), m]])
    cbias = (f_b1 @ f_w2 - f_w2.sum(axis=0) + f_b2).astype(F32)       # [64]
    gbias = (g_b1 @ g_w2 - g_w2.sum(axis=0) + g_b2).astype(F32)       # [64]
    w = dict(
        w1n=blk(w9).astype(F32),             # [18,128]  (zb = +z1)
        w12=blk(w9 @ f_w2).astype(F32),      # [18,128]
        w2p=blk(f_w2).astype(F32),           # [128,128]
        g1n=g_w1.astype(F32),                # [67,64]
        g12=(g_w1 @ g_w2).astype(F32),       # [67,64]
        g2=g_w2.astype(F32),                 # [64,64]
        nbias1=np.tile(-f_b1, 2).astype(F32).reshape(128, 1),
        cbias=cbias.reshape(64, 1),
        ngb1=(-g_b1).astype(F32).reshape(64, 1),
        pgb1=g_b1.astype(F32).reshape(64, 1),
        gbias=gbias.reshape(64, 1),
        gbm1=(gbias - 1.0).reshape(64, 1),
    )
    import ml_dtypes
    wpack = np.zeros((128, WCOL), dtype=ml_dtypes.bfloat16)
    for name, (p0, p1, c0, cn) in WSLOTS.items():
        wpack[p0:p1, c0:c0 + cn] = w[name]
    bpack = np.zeros((128, BCOL), dtype=F32)
    for name, (p, c0, cn) in BSLOTS.items():
        bpack[:p, c0:c0 + cn] = w[name]
    w["wpack"] = wpack
    w["bpack"] = bpack
    return w


def _bf(v):
    import ml_dtypes
    return np.asarray(v).astype(ml_dtypes.bfloat16).astype(F32)


# --------------------------------------------------------------------------
# numpy model of the device program (for validation; mimics bf16 rounding)
# --------------------------------------------------------------------------

def _numpy_device(feats, xnode, w, tiles, ncw):
    G = len(tiles) // 2
    aggr = np.zeros((128, ncw), dtype=F32)
    for g in range(G):
        f = _bf(feats[0:18, g * GRP:(g + 1) * GRP])
        zb = w["w1n"].T @ f                                  # +z1
        r = _bf(np.maximum(-zb + w["nbias1"], 0))
        e = _bf(np.exp(-r))
        s = _bf(r + e)
        ms = w["w12"].T @ f + w["w2p"].T @ s
        for j in (0, 1):
            t, k = tiles[2 * g + j]
            dst = aggr[:, k * TILE:(k + 1) * TILE]
            src = _bf(ms[:, j * TILE:(j + 1) * TILE])
            if t == 0:
                dst[:] = src
            else:
                np.maximum(dst, src, out=dst)
    a64 = np.maximum(aggr[0:64], aggr[64:128])
    u_in = np.empty((67, ncw), dtype=F32)
    u_in[0:64] = _bf(np.maximum(a64 + w["cbias"], 0))
    u_in[64:67] = _bf(xnode)
    out = np.empty((64, ncw), dtype=F32)
    for i in range(ncw // TILE):
        ui = u_in[:, i * TILE:(i + 1) * TILE]
        zg = w["g1n"].T @ ui
        rg = _bf(np.maximum(-zg + w["ngb1"], 0))
        y2 = _bf(np.exp(zg + w["pgb1"]))
        sg = _bf(np.minimum(y2, 1.0) + rg)
        o2 = w["g12"].T @ ui + w["g2"].T @ sg
        y = _bf(np.exp(o2 + w["gbias"]))
        vf2 = _bf(np.maximum(o2 + w["gbm1"], -1.0))
        out[:, i * TILE:(i + 1) * TILE] = _bf(np.minimum(y, 1.0) + vf2)
    return out        # [64, ncw] (bf16-rounded values)


# --------------------------------------------------------------------------
# bass program
# --------------------------------------------------------------------------

def _import_concourse():
    try:
        import concourse.bass  # noqa: F401
    except ImportError:
        sys.path.insert(0, "/opt/trn_rl_repo")


def _install_ntff_shim():
    """Provide antenv.axon_hooks (missing in this image) so that
    run_bass_kernel_spmd(trace=True) can capture NTFF profiles."""
    import contextlib
    import ctypes
    import types

    if "antenv.axon_hooks" in sys.modules:
        return
    so_path = "/opt/axon/libaxon_pjrt.so"
    if not os.path.exists(so_path):
        return
    lib = ctypes.CDLL(so_path)
    if not hasattr(lib, "axon_start_nrt_profile"):
        return
    lib.axon_start_nrt_profile.argtypes = [ctypes.POINTER(ctypes.c_int64),
                                           ctypes.c_size_t]
    lib.axon_start_nrt_profile.restype = ctypes.c_int64
    lib.axon_stop_nrt_profile.argtypes = [ctypes.c_char_p]
    lib.axon_stop_nrt_profile.restype = ctypes.c_int64

    @contextlib.contextmanager
    def _hook(output_dir, device_ids):
        import jax
        jax.devices()
        if device_ids:
            ids = (ctypes.c_int64 * len(device_ids))(*device_ids)
            rc = lib.axon_start_nrt_profile(ids, len(device_ids))
        else:
            rc = lib.axon_start_nrt_profile(None, 0)
        if rc != 0:
            raise RuntimeError(f"axon_start_nrt_profile rc={rc}")
        try:
            yield
        finally:
            n = lib.axon_stop_nrt_profile(str(output_dir).encode())
            print(f"ntff profile: {n} file(s) -> {output_dir}",
                  file=sys.stderr)

    mod = types.ModuleType("antenv.axon_hooks")
    mod.get_axon_ntff_profile_hook = lambda: _hook
    mod.set_axon_ntff_profile_hook = lambda h: None
    sys.modules["antenv.axon_hooks"] = mod


def _dep(from_inst, to_inst, reason):
    from concourse.tile import add_dep_helper
    a = getattr(from_inst, "ins", from_inst)
    b = getattr(to_inst, "ins", to_inst)
    add_dep_helper(a, b, reason=reason)


def _build_nc(tiles, S, ncw):
    _import_concourse()
    import concourse.bass as bass
    import concourse.tile as tile
    import concourse.tile_sem_assignment as _tsa
    from concourse import mybir

    # One DMAHW bookkeeping lane: HWDGE transfers share a FIFO proc, so
    # DMA-vs-DMA ordering (slot WAW) needs no extra sync wait.
    _tsa.NUM_HWDGE_SEMS = 1

    f32 = mybir.dt.float32
    bf16 = mybir.dt.bfloat16
    AF = mybir.ActivationFunctionType
    ALU = mybir.AluOpType
    nc = bass.Bass()

    G = len(tiles) // 2
    S_pad = ((S + SUP - 1) // SUP) * SUP
    n_sup = S_pad // SUP
    n_nt = ncw // TILE                       # node tiles

    feats_d = nc.dram_tensor("feats", [50, S_pad], bf16, kind="ExternalInput")
    xnode_d = nc.dram_tensor("xnode", [3, ncw], bf16, kind="ExternalInput")
    wpack_d = nc.dram_tensor("wpack", [128, WCOL], bf16, kind="ExternalInput")
    bpack_d = nc.dram_tensor("bpack", [128, BCOL], f32, kind="ExternalInput")
    out_d = nc.dram_tensor("out", [64, ncw], bf16, kind="ExternalOutput")

    # node-phase lead-in chunks (4 tiles each) and the edge-group after which
    # each chunk's aggr columns are final (chunk 0 = blocks 0-3 is last)
    n_ck = (n_nt + 3) // 4
    ck_last = []
    for c in range(n_ck):
        blocks = set(range(4 * c, min(4 * c + 4, n_nt)))
        last = 0
        for j, (t, k) in enumerate(tiles):
            if k in blocks:
                last = j // 2
        ck_last.append(last)

    with tile.TileContext(nc) as tc:
        with (
            tc.tile_pool(name="const", bufs=1) as cpool,
            tc.tile_pool(name="aggr", bufs=1) as apool,
            tc.tile_pool(name="feats", bufs=2) as fpool,
            tc.tile_pool(name="rm", bufs=3) as rmpool,
            tc.tile_pool(name="et", bufs=2) as etpool,
            tc.tile_pool(name="st", bufs=2) as stpool,
            tc.tile_pool(name="gwork", bufs=1) as gpool,
            tc.tile_pool(name="nrg", bufs=3) as nrgpool,
            tc.tile_pool(name="ny2", bufs=2) as ny2pool,
            tc.tile_pool(name="ny", bufs=3) as nypool,
            tc.tile_pool(name="nsg", bufs=2) as nsgpool,
            tc.tile_pool(name="nvf", bufs=2) as nvfpool,
            tc.tile_pool(name="psum_z", bufs=2, space="PSUM") as pz,
            tc.tile_pool(name="psum_m", bufs=4, space="PSUM") as pm,
        ):
            wsb = cpool.tile([128, WCOL], bf16, name="wsb")
            wdma = nc.sync.dma_start(wsb[:], wpack_d[:])
            bsb = cpool.tile([128, BCOL], f32, name="bsb")
            bdma = nc.sync.dma_start(bsb[:], bpack_d[:])
            w = {name: wsb[p0:p1, c0:c0 + cn]
                 for name, (p0, p1, c0, cn) in WSLOTS.items()}
            w.update({name: bsb[0:p, c0:c0 + cn]
                      for name, (p, c0, cn) in BSLOTS.items()})
            # ACT/DVE-side absorbers: observe the bias DMA once.
            tabs = cpool.tile([1, 8], f32, name="tabs")
            ta0 = nc.scalar.activation(tabs[0:1, 0:1], bsb[0:1, 0:1], AF.Copy)
            _dep(ta0, bdma, "ACT observes bias DMA")
            vscr = cpool.tile([1, 8], f32, name="vscr")
            tv0 = nc.vector.tensor_copy(vscr[0:1, 0:1], bsb[0:1, 0:1])
            _dep(tv0, bdma, "DVE observes bias DMA")

            aggr = apool.tile([128, ncw], bf16)
            u_in = gpool.tile([67, ncw], bf16, tag="u_in")
            ah = gpool.tile([64, ncw], bf16, tag="ah")
            out_sb = gpool.tile([64, ncw], bf16, tag="out_sb")

            # ---- feats superblock staging: rows 0-17 feed the w1n (zb)
            # stream in array rows 0-31; a second copy at partitions 32-49
            # feeds the w12 (ms-init) stream in array rows 32-63 so both
            # matmuls run concurrently in different row groups.
            sup_tiles = [None] * n_sup
            sup_dmas = [None] * n_sup
            def stage_sup(i):
                st_ = fpool.tile([50, SUP], bf16, tag="feats_sup")
                d = nc.sync.dma_start(st_[:],
                                      feats_d[:, i * SUP:(i + 1) * SUP])
                sup_tiles[i] = st_
                sup_dmas[i] = d
            for i in range(min(2, n_sup)):
                stage_sup(i)

            def fcols(g, band):
                c0 = g * GRP
                st_ = sup_tiles[c0 // SUP]
                fo = c0 % SUP
                if band == 0:
                    return st_[0:18, fo:fo + GRP]
                return st_[32:50, fo:fo + GRP]

            def emit_zb(g, off):
                """one 512-col zb matmul (array rows 0-31)."""
                fa = fcols(g, 0)
                if off == 0:
                    zbt = pz.tile([128, GRP], f32, tag="zb")
                    emit_zb.cur = zbt
                zbt = emit_zb.cur
                mm = nc.tensor.matmul(zbt[:, off:off + TILE], w["w1n"],
                                      fa[:, off:off + TILE], start=True,
                                      stop=True)
                return zbt, mm

            def emit_ms(g, off):
                """one 512-col ms-init matmul (array rows 32-63).  Each
                512-col half gets its own PSUM tile so the aggregate-max of
                half 0 can start as soon as half 0's w2 matmul stops."""
                fa = fcols(g, 1)
                mst = pm.tile([128, TILE], f32, tag="ms")
                mm = nc.tensor.matmul(mst[:], w["w12"],
                                      fa[:, off:off + TILE], start=True,
                                      stop=False)
                return mst, mm

            def emit_zbms(gz, gm, after=None):
                """interleaved concurrent pairs: zb(gz) in array rows 0-31
                overlaps ms-init(gm) in rows 32-63.  The explicit PE chain
                pins the scheduler to this order (alternating row groups so
                adjacent matmuls execute concurrently) and lets the w2
                twin's sup-DMA wait cover these reads."""
                zbt = None
                msts = []
                prev = after
                for off in (0, TILE):
                    if gz is not None:
                        zbt, mm = emit_zb(gz, off)
                        if prev is not None:
                            _dep(mm, prev, "pin PE order")
                        prev = mm
                    if gm is not None:
                        mst, mm = emit_ms(gm, off)
                        if prev is not None:
                            _dep(mm, prev, "pin PE order")
                        prev = mm
                        msts.append(mst)
                return zbt, msts

            def emit_re(g, zbt, prev_e):
                """r = relu(-zb-b1) [ACT], e = exp(-r) [ACT].

                r is chained after the previous group's e: that e already
                waits on the DVE s-op releasing the rm slot r reuses (rm
                bufs=3 vs et bufs=2 alignment), so r keeps only its PE wait.
                """
                rm = rmpool.tile([128, GRP], bf16, tag="rm")
                et = etpool.tile([128, GRP], bf16, tag="et")
                ri = nc.scalar.activation(rm[:], zbt[:], AF.Relu,
                                          bias=w["nbias1"], scale=-1.0)
                if prev_e is not None:
                    _dep(ri, prev_e, "rm slot WAR covered by prev e wait")
                ei = nc.scalar.activation(et[:], rm[:], AF.Exp, scale=-1.0)
                return rm, et, ei

            def emit_s(g, rm, et, prev_agg):
                """s = r + e [DVE, bf16 2x] into a PE-only tile; chained
                after the latest aggmax so its st-slot PE-WAR is covered."""
                st_ = stpool.tile([128, GRP], bf16, tag="st")
                si = nc.vector.tensor_add(st_[:], rm[:], et[:])
                if prev_agg is not None:
                    _dep(si, prev_agg, "st slot WAR covered by aggmax wait")
                return st_

            def emit_w2(g, msts, st_):
                mm = []
                for j, off in enumerate((0, TILE)):
                    mm.append(nc.tensor.matmul(
                        msts[j][:], w["w2p"],
                        st_[:, off:off + TILE], start=False, stop=True))
                return mm

            def emit_aggmax(g, msts):
                outs = []
                for j in (0, 1):
                    t, k = tiles[2 * g + j]
                    dst = aggr[:, k * TILE:(k + 1) * TILE]
                    src = msts[j][:]
                    if t == 0:
                        outs.append(nc.vector.tensor_copy(dst, src))
                    else:
                        outs.append(nc.vector.tensor_max(dst, dst, src))
                return outs

            # ---- node-phase lead-in (per 4-tile chunk): move odd-round half
            # down, fold max, relu+cbias into u_in; emitted as soon as the
            # chunk's aggr columns are final so it hides under the edge phase
            def emit_chunk(c):
                c0 = 4 * c * TILE
                cw = min(ncw - c0, 4 * TILE)
                d = nc.sync.dma_start(ah[:, c0:c0 + cw],
                                      aggr[64:128, c0:c0 + cw])
                tvc = nc.vector.tensor_copy(vscr[0:1, 1:2], bsb[0:1, 0:1])
                _dep(tvc, d, "DVE absorbs fold DMA dep")
                fo = nc.vector.tensor_max(ah[:, c0:c0 + cw],
                                          aggr[0:64, c0:c0 + cw],
                                          ah[:, c0:c0 + cw])
                _dep(fo, tvc, "order after absorber")
                ur = nc.scalar.activation(u_in[0:64, c0:c0 + cw],
                                          ah[:, c0:c0 + cw], AF.Relu,
                                          bias=w["cbias"], scale=1.0)
                return ur

            # =========== edge phase ===========
            zb_t = {}
            ms_t = {}
            rm_t = {}
            et_t = {}
            st_t = {}
            zb_t[0], ms_t[0] = emit_zbms(0, 0)
            zb_t[1], _ = emit_zbms(1, None)
            rm_t[0], et_t[0], prev_e = emit_re(0, zb_t[0], None)
            st_t[0] = emit_s(0, rm_t[0], et_t[0], None)
            prev_agg = None

            chunks_done = set()
            chunk_insts = {}
            for g in range(G):
                mm_e = emit_w2(g, ms_t[g], st_t.pop(g))
                rm_t.pop(g)
                et_t.pop(g)
                # if groups g+1/g+2 (read next on PE) start a new superblock,
                # absorb that DMA's wait on the wait-free w2 twin
                c_nxt = (g + 2) * GRP
                if g + 2 < G and c_nxt % SUP == 0 \
                        and sup_dmas[c_nxt // SUP] is not None:
                    _dep(mm_e[1], sup_dmas[c_nxt // SUP],
                         "sup prefetch via w2 twin")
                prev_agg = emit_aggmax(g, ms_t.pop(g))[-1]
                # prefetch the superblock that group g+3 will read
                c3 = (g + 3) * GRP
                if g + 3 < G and c3 % SUP == 0 and c3 // SUP < n_sup \
                        and sup_tiles[c3 // SUP] is None:
                    stage_sup(c3 // SUP)
                gz = g + 2 if g + 2 < G else None
                gm = g + 1 if g + 1 < G else None
                if gz is not None or gm is not None:
                    zbt, mst = emit_zbms(gz, gm, after=mm_e[1])
                    if gz is not None:
                        zb_t[gz] = zbt
                    if gm is not None:
                        ms_t[gm] = mst
                if g + 1 < G:
                    rm_t[g + 1], et_t[g + 1], prev_e = emit_re(
                        g + 1, zb_t[g + 1], prev_e)
                    st_t[g + 1] = emit_s(g + 1, rm_t[g + 1], et_t[g + 1],
                                         prev_agg)
                # early node-phase chunks once their blocks are final
                for c in range(1, n_ck):
                    if c not in chunks_done and ck_last[c] == g:
                        chunks_done.add(c)
                        chunk_insts[c] = emit_chunk(c)

            xdma = nc.sync.dma_start(u_in[64:67, :], xnode_d[:])
            chunk_insts[0] = emit_chunk(0)
            for c in range(1, n_ck):
                if c not in chunks_done:
                    chunk_insts[c] = emit_chunk(c)

            # absorber matmuls: observe the chunk-0 relu (ACT) and the xnode
            # DMA so the first node matmuls keep a single sem wait
            kwt1 = pz.tile([64, TILE], f32, tag="zb", name="kwt1")
            kw1 = nc.tensor.matmul(kwt1[:], wsb[0:64, 0:64],
                                   wsb[0:64, 0:TILE], start=True, stop=True)
            _dep(kw1, chunk_insts[0], "observe chunk-0 relu")
            kwt2 = pz.tile([64, TILE], f32, tag="zb", name="kwt2")
            kw2 = nc.tensor.matmul(kwt2[:], wsb[0:64, 0:64],
                                   wsb[0:64, 0:TILE], start=True, stop=True)
            _dep(kw2, xdma, "absorb xnode DMA wait")

            # =========== node phase ===========
            # per tile: zg = g1n@u; rg = relu(-zg-gb1); y2 = exp(zg+gb1);
            # sg = min(y2,1)+rg; o2 = g12@u + g2@sg; y = exp(o2+gbias);
            # vf2 = max(o2+gbias-1, -1); out = min(y,1)+vf2.
            # Explicit deps keep every instruction at <=1 sync wait.
            zg_t = {}
            o2_t = {}

            def emit_nmm1(i, prev_g2sg):
                ui = u_in[:, i * TILE:(i + 1) * TILE]
                zg = pz.tile([64, TILE], f32, tag="zb")
                nc.tensor.matmul(zg[:], w["g1n"], ui, start=True, stop=True)
                o2 = pm.tile([64, TILE], f32, tag="ms")
                o2i = nc.tensor.matmul(o2[:], w["g12"], ui, start=True,
                                       stop=False)
                if prev_g2sg is not None:
                    _dep(o2i, prev_g2sg, "pm slot DVE-WAR covered by g2sg")
                return zg, o2

            def emit_nact(i, prev_y2):
                rg = nrgpool.tile([64, TILE], bf16, tag="nrg")
                y2 = ny2pool.tile([64, TILE], bf16, tag="ny2")
                rgi = nc.scalar.activation(rg[:], zg_t[i][:], AF.Relu,
                                           bias=w["ngb1"], scale=-1.0)
                if prev_y2 is not None:
                    _dep(rgi, prev_y2, "nrg slot WAR covered by prev y2")
                y2i = nc.scalar.activation(y2[:], zg_t[i][:], AF.Exp,
                                           bias=w["pgb1"], scale=1.0)
                _dep(y2i, rgi, "share zg PE wait")
                return rg, y2, y2i

            def emit_nsg(i, rg, y2, prev_vf2):
                sg = nsgpool.tile([64, TILE], bf16, tag="nsg")
                sgi = nc.vector.scalar_tensor_tensor(
                    sg[:], y2[:], 1.0, rg[:], op0=ALU.min, op1=ALU.add)
                if prev_vf2 is not None:
                    _dep(sgi, prev_vf2, "nsg slot PE-WAR covered by vf2")
                return sg

            zg_t[0], o2_t[0] = emit_nmm1(0, None)
            rg_c, y2_c, y2i_c = emit_nact(0, None)
            sg_t = {0: emit_nsg(0, rg_c, y2_c, None)}
            prev_y2i = y2i_c
            prev_vf2 = None

            for i in range(n_nt):
                o2 = o2_t.pop(i)
                mm_o2 = nc.tensor.matmul(o2[:], w["g2"], sg_t.pop(i)[:],
                                         start=False, stop=True)
                if i + 1 < n_nt:
                    zg_t[i + 1], o2_t[i + 1] = emit_nmm1(i + 1, mm_o2)
                    rg_c, y2_c, y2i_c = emit_nact(i + 1, prev_y2i)
                    prev_y2i = y2i_c
                y = nypool.tile([64, TILE], bf16, tag="ny")
                yi = nc.scalar.activation(y[:], o2[:], AF.Exp,
                                          bias=w["gbias"], scale=1.0)
                _dep(yi, prev_y2i, "ny slot DVE-WAR covered by y2 wait")
                vf2 = nvfpool.tile([64, TILE], bf16, tag="nvf")
                vf2i = nc.vector.tensor_scalar(vf2[:], o2[:], w["gbm1"], -1.0,
                                               ALU.add, ALU.max)
                nc.vector.scalar_tensor_tensor(
                    out_sb[:, i * TILE:(i + 1) * TILE], y[:], 1.0, vf2[:],
                    op0=ALU.min, op1=ALU.add)
                if i + 1 < n_nt:
                    sg_t[i + 1] = emit_nsg(i + 1, rg_c, y2_c, vf2i)
                nc.sync.dma_start(out_d[:, i * TILE:(i + 1) * TILE],
                                  out_sb[:, i * TILE:(i + 1) * TILE])

    return nc


def _set_waits(inst, kept):
    """Replace an instruction's sync waits.  inst.sync_info returns a copy,
    so rebuild fresh SyncWait/SyncInfo objects and assign them back to the
    instruction."""
    import bass_rust
    news = [bass_rust.SyncWait(sync_type=x.sync_type, id=x.id,
                               wait_mode=x.wait_mode, ant_name=x.ant_name,
                               wait_value=x.wait_value, wait_reg=x.wait_reg)
            for x in kept]
    si = inst.sync_info
    ups = [bass_rust.SyncUpdate(sync_type=u.sync_type, id=u.id,
                                ant_name=u.ant_name, update_value=u.update_value)
           if False else u for u in list(si.on_update)]
    inst.sync_info = bass_rust.SyncInfo(on_wait=news, on_update=ups)


def _prune_waits(nc):
    """ISA structs carry at most one sync wait. Drop provably-redundant
    waits Tile emitted (same-engine self-waits on strict-FIFO engines;
    DMA-vs-DMA ordering subsumed by compute waits; drain-tail waits)."""
    n1 = n2 = 0
    for b in nc.m.functions[0].blocks:
        for i in b.instructions:
            si = i.sync_info
            if si is None or not si.on_wait or len(si.on_wait) < 2:
                continue
            nm = type(i).__name__
            waits = list(si.on_wait)
            if nm == "InstDrain":
                dma_w = [x for x in waits if x.ant_name.startswith("DMAHW")]
                _set_waits(i, dma_w[-1:] if dma_w else waits[-1:])
                continue
            if nm == "InstMatmult":
                act_w = [x for x in waits
                         if x.ant_name.startswith("Activation")]
                dma_w = [x for x in waits if x.ant_name.startswith("DMAHW")]
                if act_w and dma_w and len(act_w) + len(dma_w) == len(waits):
                    # sup-boundary zb matmul: its ACT slot-WAR (r read of the
                    # zb slot two groups back) is transitively enforced by the
                    # preceding w2 matmul's DVE wait (w2 <- s <- e <- r, ACT
                    # strict FIFO), so only the feats-DMA RAW must remain.
                    n1 += len(act_w)
                    _set_waits(i, dma_w)
                    continue
            if nm == "InstDMACopy":
                kept = [x for x in waits
                        if not (x.ant_name.startswith("DMAHW") or
                                x.ant_name.startswith("DMASW"))]
                if kept and len(kept) < len(waits):
                    n2 += len(waits) - len(kept)
                    _set_waits(i, kept)
                continue
            own = str(i.engine).split(".")[-1]
            kept = [x for x in waits
                    if x.ant_name.rsplit("_", 1)[0] != own]
            if len(kept) < len(waits):
                n1 += len(waits) - len(kept)
                _set_waits(i, kept)
    return n1, n2


# --------------------------------------------------------------------------
# entry points
# --------------------------------------------------------------------------

def _prepare(x, pos, edge_index, f_w1, f_b1, f_w2, f_b2,
             g_w1, g_b1, g_w2, g_b2):
    x = np.asarray(x, F32)
    pos = np.asarray(pos, F32)
    src = np.asarray(edge_index[0]).astype(np.int64)
    dst = np.asarray(edge_index[1]).astype(np.int64)
    cores = _core_layouts(edge_index)
    tiles, S, ncw = _tile_plan(cores)
    S_pad = ((S + SUP - 1) // SUP) * SUP
    packs = []
    for c, core in enumerate(cores):
        feats, xnode = _pack_core(core, tiles, S_pad, ncw, x, pos, src, dst)
        xnode[:, :NCN] = x[core["order"] + c * NCN].T
        packs.append((feats, xnode))
    w = _weights(np.asarray(f_w1, F32), np.asarray(f_b1, F32),
                 np.asarray(f_w2, F32), np.asarray(f_b2, F32),
                 np.asarray(g_w1, F32), np.asarray(g_b1, F32),
                 np.asarray(g_w2, F32), np.asarray(g_b2, F32))
    return cores, tiles, S_pad, ncw, packs, w


def _finalize(results, cores, x, g_w1, g_b1, g_w2, g_b2):
    """results: list of [64, ncw] per core -> full [N, 64] output."""
    out = np.empty((N, 64), dtype=F32)
    for c, core in enumerate(cores):
        out[core["order"] + c * NCN] = np.asarray(
            results[c], F32)[:, :NCN].T
    empties = np.concatenate([c["empty"] for c in cores])
    if empties.size:
        def celu(v):
            return np.maximum(v, 0) + np.minimum(0, np.expm1(np.minimum(v, 0)))
        u_in = np.concatenate(
            [np.zeros((empties.size, 64), F32), x[empties]], axis=1)
        u = celu(u_in @ g_w1 + g_b1)
        out[empties] = celu(u @ g_w2 + g_b2).astype(F32)
    return out


def kernel(x, pos, edge_index, f_w1, f_b1, f_w2, f_b2,
           g_w1, g_b1, g_w2, g_b2, _debug_numpy=False, _trace=False):
    x = np.asarray(x, F32)
    pos = np.asarray(pos, F32)
    cores, tiles, S_pad, ncw, packs, w = _prepare(
        x, pos, edge_index, f_w1, f_b1, f_w2, f_b2, g_w1, g_b1, g_w2, g_b2)

    if _debug_numpy:
        results = [_numpy_device(f, xn, w, tiles, ncw) for (f, xn) in packs]
        return _finalize(results, cores, x, np.asarray(g_w1, F32),
                         np.asarray(g_b1, F32), np.asarray(g_w2, F32),
                         np.asarray(g_b2, F32))

    _import_concourse()
    run_kwargs = {}
    if _trace:
        _install_ntff_shim()
        import concourse.bass_utils as _bu
        _bu.upload_artifacts = lambda tmpdir: f"file://{tmpdir}"
        import tempfile
        trace_dir = tempfile.mkdtemp(prefix="bass_trace_")
        run_kwargs = dict(tmpdir=trace_dir)
        kernel._last_trace_dir = trace_dir
    from concourse.bass_utils import run_bass_kernel_spmd

    import ml_dtypes
    bf = ml_dtypes.bfloat16
    nc = _build_nc(tiles, S_pad, ncw)
    _prune_waits(nc)
    in_maps = [{"feats": feats.astype(bf), "xnode": xnode.astype(bf),
                "wpack": w["wpack"], "bpack": w["bpack"]}
               for (feats, xnode) in packs]
    res = run_bass_kernel_spmd(nc, in_maps, list(range(CORES)), trace=_trace,
                               **run_kwargs)
    results = [res.results[c]["out"] for c in range(CORES)]
    out = _finalize(results, cores, x, np.asarray(g_w1, F32),
                    np.asarray(g_b1, F32), np.asarray(g_w2, F32),
                    np.asarray(g_b2, F32))
    if _trace:
        kernel._last_exec_time_ns = res.exec_time_ns
        kernel._last_mean_exec_time_ns = res.mean_exec_time_ns
    return out


# revision 26
# speedup vs baseline: 1.5659x; 1.0025x over previous
"""Trainium2 Bass kernel for a GNN message-passing layer.

Reference semantics (per edge e = (src j, dst i)):
    m_in  = [x_j, pos_j - pos_i]                 # [E, 6]
    h     = celu(m_in @ f_w1 + f_b1)             # [E, 64]
    msg   = relu(h @ f_w2 + f_b2)                # [E, 64]
    aggr  = segment_max(msg, dst, N); empty -> 0 # [N, 64]
    u     = celu([aggr, x] @ g_w1 + g_b1)
    out   = celu(u @ g_w2 + g_b2)                # [N, 64]

Sharding: nodes split into 8 contiguous ranges (6250/core); each core gets the
edges whose dst is in its range, so segment-max is local.  Host does
index-only work (degree-sort, round layout, gather); device does every FLOP.

Device program (v2): celu decomposed as celu(z) = relu(-z) + exp(-relu(-z))
+ z - 1.  Per 1024-column group (2 edges stacked per column):
  zb = w9@f (PSUM), then either
    A-path: r = ACT.Relu(-zb-b1), e = ACT.Exp(-r); ms += w2@r + w2@e
    D-path: m = DVE.ts(zb+b1 min 0) (= -r), e = ACT.Exp(m); ms += (-w2)@m + w2@e
  ms also accumulates w12@f (the linear z term), then DVE tensor-max into a
  bf16 running aggregate (relu+bias deferred past the max).
The PE stream is software-pipelined depth-2 (w2-streams of group g run while
zb of g+2 and ms-init of g+1 are computed) so the tensor engine never waits
on ACT; a gap-free warmup burst un-throttles the PE HAM clock gate
(1.2 -> 2.4 GHz) at kernel start and keep-warm dummies span the node-phase
lead-in.
"""

import math
import os
import sys

import numpy as np

N = 50000
E = 1600000
CORES = 8
NCN = N // CORES            # nodes per core
TILE = 512                  # fp32 matmul moving free dim / one PSUM bank
GRP = 1024                  # group width (columns) = 2 tiles
SUP = 4096                  # feats DMA staging superblock (columns) = 4 groups
F32 = np.float32
DPAT = 3                    # every DPAT-th group takes the DVE (m) path


# --------------------------------------------------------------------------
# host-side layout (index work only)
# --------------------------------------------------------------------------

def _core_layouts(edge_index):
    """Per-core node ordering + degree-sorted CSR of local edges."""
    dst = np.asarray(edge_index[1])
    cores = []
    for c in range(CORES):
        lo, hi = c * NCN, (c + 1) * NCN
        eids = np.nonzero((dst >= lo) & (dst < hi))[0]
        ldst = (dst[eids] - lo).astype(np.int64)
        deg = np.bincount(ldst, minlength=NCN)
        order = np.argsort(-deg, kind="stable")         # node ranks
        rank = np.empty(NCN, np.int64)
        rank[order] = np.arange(NCN)
        perm = np.argsort(rank[ldst], kind="stable")
        es = eids[perm]                                  # edges sorted by rank
        deg_s = deg[order]
        row_start = np.zeros(NCN + 1, np.int64)
        np.cumsum(deg_s, out=row_start[1:])
        cores.append(dict(es=es, deg_s=deg_s, row_start=row_start,
                          order=order, empty=order[deg_s == 0] + lo))
    return cores


def _tile_plan(cores):
    """Shared (SPMD-uniform) tile plan at 512-column granularity.

    tiles: list of (pair_round t, node_block k); tile covers node ranks
    [512k, 512k+512) at rounds (2t, 2t+1).  Flat consecutive pairs of tiles
    form 1024-column groups (groups may straddle rounds; the aggregate-max
    is per-tile anyway).
    """
    rmax = max(int(c["deg_s"][0]) for c in cores)
    n_pairs = (rmax + 1) // 2
    tiles = []
    for t in range(n_pairs):
        w = max(int(np.searchsorted(-c["deg_s"], -(2 * t), side="left"))
                for c in cores)      # max over cores of #nodes with deg > 2t
        if t == 0:
            w = NCN                  # every aggr column gets initialized
        for k in range(max(1, (w + TILE - 1) // TILE)):
            tiles.append((t, k))
    if len(tiles) % 2:
        assert tiles[-1][0] > 0
        tiles.append(tiles[-1])      # dup: max is idempotent, not first-touch
    S = TILE * len(tiles)
    ncw = TILE * ((NCN + TILE - 1) // TILE)
    return tiles, S, ncw


def _pack_core(core, tiles, S, ncw, x, pos, src, dst):
    """Build one core's slot->edge assignment and gather features."""
    es, deg_s, row_start = core["es"], core["deg_s"], core["row_start"]
    ncols = len(tiles) * TILE
    nvec = np.tile(np.arange(TILE, dtype=np.int64), len(tiles))  # col in tile
    kvec = np.repeat([k for (_, k) in tiles], TILE)
    tvec = np.repeat([t for (t, _) in tiles], TILE)
    node = kvec * TILE + nvec                    # node rank targeted by column

    safe_node = np.minimum(node, NCN - 1)
    ecap = len(es) - 1
    first_edge = es[np.minimum(row_start[safe_node], ecap)]  # dup fallback
    bad = (node >= NCN) | (deg_s[safe_node] == 0)
    first_edge = np.where(bad, es[0], first_edge)

    def round_edges(r):
        has = (~bad) & (deg_s[safe_node] > r)
        idx = np.minimum(row_start[safe_node] + np.where(has, r, 0), ecap)
        return np.where(has, es[idx], first_edge)

    a_e = round_edges(2 * tvec)        # vectorized: r differs per column
    b_e = round_edges(2 * tvec + 1)

    # rows 0-17: features for the w1n (zb) stream; rows 32-49: the same
    # features again for the w12 (ms-init) stream, so each superblock is a
    # single rectangular DMA and the two matmul streams read disjoint
    # partition bands (array rows 0-31 / 32-63, concurrent row groups)
    feats = np.zeros((50, S), dtype=F32)
    for half, eids in ((0, a_e), (9, b_e)):
        s, d = src[eids], dst[eids]
        feats[half + 0:half + 3, :ncols] = x[s].T
        feats[half + 3:half + 6, :ncols] = pos[s].T
        feats[half + 6:half + 9, :ncols] = pos[d].T
    feats[32:50] = feats[0:18]

    xnode = np.zeros((3, ncw), dtype=F32)
    xnode[:, :NCN] = x[core["order"] + 0].T      # caller adds core offset
    return feats, xnode


# column layouts of the packed weight tensors (bf16 matmul operands; PE runs
# fp32 at 1/4 rate, bf16 streams 1 col/cycle with f32 PSUM accumulation).
# w12 lives at partitions 32-49 so its matmuls run in array rows 32-63,
# concurrent with the w1n (rows 0-31) matmuls.
WSLOTS = dict(w1n=(0, 18, 0, 128), w12=(32, 50, 128, 128),
              w2p=(0, 128, 256, 128), g1n=(0, 67, 384, 64),
              g12=(0, 67, 448, 64), g2=(0, 64, 512, 64))
WCOL = 576
BSLOTS = dict(nbias1=(128, 0, 1), cbias=(64, 1, 1), ngb1=(64, 2, 1),
              pgb1=(64, 3, 1), gbias=(64, 4, 1), gbm1=(64, 5, 1))
BCOL = 8


def _weights(f_w1, f_b1, f_w2, f_b2, g_w1, g_b1, g_w2, g_b2):
    w9 = np.concatenate([f_w1[0:3], f_w1[3:6], -f_w1[3:6]], axis=0)  # [9,64]
    blk = lambda m: np.block([[m, np.zeros_like(m)], [np.zeros_like(m), m]])
    cbias = (f_b1 @ f_w2 - f_w2.sum(axis=0) + f_b2).astype(F32)       # [64]
    gbias = (g_b1 @ g_w2 - g_w2.sum(axis=0) + g_b2).astype(F32)       # [64]
    w = dict(
        w1n=blk(w9).astype(F32),             # [18,128]  (zb = +z1)
        w12=blk(w9 @ f_w2).astype(F32),      # [18,128]
        w2p=blk(f_w2).astype(F32),           # [128,128]
        g1n=g_w1.astype(F32),                # [67,64]
        g12=(g_w1 @ g_w2).astype(F32),       # [67,64]
        g2=g_w2.astype(F32),                 # [64,64]
        nbias1=np.tile(-f_b1, 2).astype(F32).reshape(128, 1),
        cbias=cbias.reshape(64, 1),
        ngb1=(-g_b1).astype(F32).reshape(64, 1),
        pgb1=g_b1.astype(F32).reshape(64, 1),
        gbias=gbias.reshape(64, 1),
        gbm1=(gbias - 1.0).reshape(64, 1),
    )
    import ml_dtypes
    wpack = np.zeros((128, WCOL), dtype=ml_dtypes.bfloat16)
    for name, (p0, p1, c0, cn) in WSLOTS.items():
        wpack[p0:p1, c0:c0 + cn] = w[name]
    bpack = np.zeros((128, BCOL), dtype=F32)
    for name, (p, c0, cn) in BSLOTS.items():
        bpack[:p, c0:c0 + cn] = w[name]
    w["wpack"] = wpack
    w["bpack"] = bpack
    return w


def _bf(v):
    import ml_dtypes
    return np.asarray(v).astype(ml_dtypes.bfloat16).astype(F32)


# --------------------------------------------------------------------------
# numpy model of the device program (for validation; mimics bf16 rounding)
# --------------------------------------------------------------------------

def _numpy_device(feats, xnode, w, tiles, ncw):
    G = len(tiles) // 2
    aggr = np.zeros((128, ncw), dtype=F32)
    for g in range(G):
        f = _bf(feats[0:18, g * GRP:(g + 1) * GRP])
        zb = w["w1n"].T @ f                                  # +z1
        r = _bf(np.maximum(-zb + w["nbias1"], 0))
        e = _bf(np.exp(-r))
        s = _bf(r + e)
        ms = w["w12"].T @ f + w["w2p"].T @ s
        for j in (0, 1):
            t, k = tiles[2 * g + j]
            dst = aggr[:, k * TILE:(k + 1) * TILE]
            src = _bf(ms[:, j * TILE:(j + 1) * TILE])
            if t == 0:
                dst[:] = src
            else:
                np.maximum(dst, src, out=dst)
    a64 = np.maximum(aggr[0:64], aggr[64:128])
    u_in = np.empty((67, ncw), dtype=F32)
    u_in[0:64] = _bf(np.maximum(a64 + w["cbias"], 0))
    u_in[64:67] = _bf(xnode)
    out = np.empty((64, ncw), dtype=F32)
    for i in range(ncw // TILE):
        ui = u_in[:, i * TILE:(i + 1) * TILE]
        zg = w["g1n"].T @ ui
        rg = _bf(np.maximum(-zg + w["ngb1"], 0))
        y2 = _bf(np.exp(zg + w["pgb1"]))
        sg = _bf(np.minimum(y2, 1.0) + rg)
        o2 = w["g12"].T @ ui + w["g2"].T @ sg
        y = _bf(np.exp(o2 + w["gbias"]))
        vf2 = _bf(np.maximum(o2 + w["gbm1"], -1.0))
        out[:, i * TILE:(i + 1) * TILE] = _bf(np.minimum(y, 1.0) + vf2)
    return out        # [64, ncw] (bf16-rounded values)


# --------------------------------------------------------------------------
# bass program
# --------------------------------------------------------------------------

def _import_concourse():
    try:
        import concourse.bass  # noqa: F401
    except ImportError:
        sys.path.insert(0, "/opt/trn_rl_repo")


def _install_ntff_shim():
    """Provide antenv.axon_hooks (missing in this image) so that
    run_bass_kernel_spmd(trace=True) can capture NTFF profiles."""
    import contextlib
    import ctypes
    import types

    if "antenv.axon_hooks" in sys.modules:
        return
    so_path = "/opt/axon/libaxon_pjrt.so"
    if not os.path.exists(so_path):
        return
    lib = ctypes.CDLL(so_path)
    if not hasattr(lib, "axon_start_nrt_profile"):
        return
    lib.axon_start_nrt_profile.argtypes = [ctypes.POINTER(ctypes.c_int64),
                                           ctypes.c_size_t]
    lib.axon_start_nrt_profile.restype = ctypes.c_int64
    lib.axon_stop_nrt_profile.argtypes = [ctypes.c_char_p]
    lib.axon_stop_nrt_profile.restype = ctypes.c_int64

    @contextlib.contextmanager
    def _hook(output_dir, device_ids):
        import jax
        jax.devices()
        if device_ids:
            ids = (ctypes.c_int64 * len(device_ids))(*device_ids)
            rc = lib.axon_start_nrt_profile(ids, len(device_ids))
        else:
            rc = lib.axon_start_nrt_profile(None, 0)
        if rc != 0:
            raise RuntimeError(f"axon_start_nrt_profile rc={rc}")
        try:
            yield
        finally:
            n = lib.axon_stop_nrt_profile(str(output_dir).encode())
            print(f"ntff profile: {n} file(s) -> {output_dir}",
                  file=sys.stderr)

    mod = types.ModuleType("antenv.axon_hooks")
    mod.get_axon_ntff_profile_hook = lambda: _hook
    mod.set_axon_ntff_profile_hook = lambda h: None
    sys.modules["antenv.axon_hooks"] = mod


def _dep(from_inst, to_inst, reason):
    from concourse.tile import add_dep_helper
    a = getattr(from_inst, "ins", from_inst)
    b = getattr(to_inst, "ins", to_inst)
    add_dep_helper(a, b, reason=reason)


def _build_nc(tiles, S, ncw):
    _import_concourse()
    import concourse.bass as bass
    import concourse.tile as tile
    import concourse.tile_sem_assignment as _tsa
    from concourse import mybir

    # One DMAHW bookkeeping lane: HWDGE transfers share a FIFO proc, so
    # DMA-vs-DMA ordering (slot WAW) needs no extra sync wait.
    _tsa.NUM_HWDGE_SEMS = 1

    f32 = mybir.dt.float32
    bf16 = mybir.dt.bfloat16
    AF = mybir.ActivationFunctionType
    ALU = mybir.AluOpType
    nc = bass.Bass()

    G = len(tiles) // 2
    S_pad = ((S + SUP - 1) // SUP) * SUP
    n_sup = S_pad // SUP
    n_nt = ncw // TILE                       # node tiles

    feats_d = nc.dram_tensor("feats", [50, S_pad], bf16, kind="ExternalInput")
    xnode_d = nc.dram_tensor("xnode", [3, ncw], bf16, kind="ExternalInput")
    wpack_d = nc.dram_tensor("wpack", [128, WCOL], bf16, kind="ExternalInput")
    bpack_d = nc.dram_tensor("bpack", [128, BCOL], f32, kind="ExternalInput")
    out_d = nc.dram_tensor("out", [64, ncw], bf16, kind="ExternalOutput")

    # node-phase lead-in chunks (4 tiles each) and the edge-group after which
    # each chunk's aggr columns are final (chunk 0 = blocks 0-3 is last)
    n_ck = (n_nt + 3) // 4
    ck_last = []
    for c in range(n_ck):
        blocks = set(range(4 * c, min(4 * c + 4, n_nt)))
        last = 0
        for j, (t, k) in enumerate(tiles):
            if k in blocks:
                last = j // 2
        ck_last.append(last)

    with tile.TileContext(nc) as tc:
        with (
            tc.tile_pool(name="const", bufs=1) as cpool,
            tc.tile_pool(name="aggr", bufs=1) as apool,
            tc.tile_pool(name="feats", bufs=2) as fpool,
            tc.tile_pool(name="rm", bufs=3) as rmpool,
            tc.tile_pool(name="et", bufs=2) as etpool,
            tc.tile_pool(name="st", bufs=2) as stpool,
            tc.tile_pool(name="gwork", bufs=1) as gpool,
            tc.tile_pool(name="nrg", bufs=3) as nrgpool,
            tc.tile_pool(name="ny2", bufs=2) as ny2pool,
            tc.tile_pool(name="ny", bufs=3) as nypool,
            tc.tile_pool(name="nsg", bufs=2) as nsgpool,
            tc.tile_pool(name="nvf", bufs=2) as nvfpool,
            tc.tile_pool(name="psum_z", bufs=2, space="PSUM") as pz,
            tc.tile_pool(name="psum_m", bufs=4, space="PSUM") as pm,
        ):
            wsb = cpool.tile([128, WCOL], bf16, name="wsb")
            wdma = nc.sync.dma_start(wsb[:], wpack_d[:])
            bsb = cpool.tile([128, BCOL], f32, name="bsb")
            bdma = nc.sync.dma_start(bsb[:], bpack_d[:])
            w = {name: wsb[p0:p1, c0:c0 + cn]
                 for name, (p0, p1, c0, cn) in WSLOTS.items()}
            w.update({name: bsb[0:p, c0:c0 + cn]
                      for name, (p, c0, cn) in BSLOTS.items()})
            # ACT/DVE-side absorbers: observe the bias DMA once.
            tabs = cpool.tile([1, 8], f32, name="tabs")
            ta0 = nc.scalar.activation(tabs[0:1, 0:1], bsb[0:1, 0:1], AF.Copy)
            _dep(ta0, bdma, "ACT observes bias DMA")
            vscr = cpool.tile([1, 8], f32, name="vscr")
            tv0 = nc.vector.tensor_copy(vscr[0:1, 0:1], bsb[0:1, 0:1])
            _dep(tv0, bdma, "DVE observes bias DMA")

            aggr = apool.tile([128, ncw], bf16)
            u_in = gpool.tile([67, ncw], bf16, tag="u_in")
            ah = gpool.tile([64, ncw], bf16, tag="ah")
            out_sb = gpool.tile([64, ncw], bf16, tag="out_sb")

            # ---- feats superblock staging: rows 0-17 feed the w1n (zb)
            # stream in array rows 0-31; a second copy at partitions 32-49
            # feeds the w12 (ms-init) stream in array rows 32-63 so both
            # matmuls run concurrently in different row groups.
            sup_tiles = [None] * n_sup
            sup_dmas = [None] * n_sup
            def stage_sup(i):
                st_ = fpool.tile([50, SUP], bf16, tag="feats_sup")
                d = nc.sync.dma_start(st_[:],
                                      feats_d[:, i * SUP:(i + 1) * SUP])
                sup_tiles[i] = st_
                sup_dmas[i] = d
            for i in range(min(2, n_sup)):
                stage_sup(i)

            def fcols(g, band):
                c0 = g * GRP
                st_ = sup_tiles[c0 // SUP]
                fo = c0 % SUP
                if band == 0:
                    return st_[0:18, fo:fo + GRP]
                return st_[32:50, fo:fo + GRP]

            def emit_zb(g, off):
                """one 512-col zb matmul (array rows 0-31)."""
                fa = fcols(g, 0)
                if off == 0:
                    zbt = pz.tile([128, GRP], f32, tag="zb")
                    emit_zb.cur = zbt
                zbt = emit_zb.cur
                mm = nc.tensor.matmul(zbt[:, off:off + TILE], w["w1n"],
                                      fa[:, off:off + TILE], start=True,
                                      stop=True)
                return zbt, mm

            def emit_ms(g, off):
                """one 512-col ms-init matmul (array rows 32-63).  Each
                512-col half gets its own PSUM tile so the aggregate-max of
                half 0 can start as soon as half 0's w2 matmul stops."""
                fa = fcols(g, 1)
                mst = pm.tile([128, TILE], f32, tag="ms")
                mm = nc.tensor.matmul(mst[:], w["w12"],
                                      fa[:, off:off + TILE], start=True,
                                      stop=False)
                return mst, mm

            def emit_zbms(gz, gm, after=None):
                """interleaved concurrent pairs: zb(gz) in array rows 0-31
                overlaps ms-init(gm) in rows 32-63.  The explicit PE chain
                pins the scheduler to this order (alternating row groups so
                adjacent matmuls execute concurrently) and lets the w2
                twin's sup-DMA wait cover these reads."""
                zbt = None
                msts = []
                prev = after
                for off in (0, TILE):
                    if gz is not None:
                        zbt, mm = emit_zb(gz, off)
                        if prev is not None:
                            _dep(mm, prev, "pin PE order")
                        prev = mm
                    if gm is not None:
                        mst, mm = emit_ms(gm, off)
                        if prev is not None:
                            _dep(mm, prev, "pin PE order")
                        prev = mm
                        msts.append(mst)
                return zbt, msts

            def emit_re(g, zbt, prev_e):
                """r = relu(-zb-b1) [ACT], e = exp(-r) [ACT].

                r is chained after the previous group's e: that e already
                waits on the DVE s-op releasing the rm slot r reuses (rm
                bufs=3 vs et bufs=2 alignment), so r keeps only its PE wait.
                """
                rm = rmpool.tile([128, GRP], bf16, tag="rm")
                et = etpool.tile([128, GRP], bf16, tag="et")
                ri = nc.scalar.activation(rm[:], zbt[:], AF.Relu,
                                          bias=w["nbias1"], scale=-1.0)
                if prev_e is not None:
                    _dep(ri, prev_e, "rm slot WAR covered by prev e wait")
                ei = nc.scalar.activation(et[:], rm[:], AF.Exp, scale=-1.0)
                return rm, et, ei

            def emit_s(g, rm, et, prev_agg):
                """s = r + e [DVE, bf16 2x] into a PE-only tile; chained
                after the latest aggmax so its st-slot PE-WAR is covered."""
                st_ = stpool.tile([128, GRP], bf16, tag="st")
                si = nc.vector.tensor_add(st_[:], rm[:], et[:])
                if prev_agg is not None:
                    _dep(si, prev_agg, "st slot WAR covered by aggmax wait")
                return st_

            def emit_w2(g, msts, st_):
                mm = []
                for j, off in enumerate((0, TILE)):
                    mm.append(nc.tensor.matmul(
                        msts[j][:], w["w2p"],
                        st_[:, off:off + TILE], start=False, stop=True))
                emit_w2.last_msts = msts
                return mm

            def emit_aggmax(g, msts):
                if msts is None:
                    msts = emit_w2.last_msts
                outs = []
                for j in (0, 1):
                    t, k = tiles[2 * g + j]
                    dst = aggr[:, k * TILE:(k + 1) * TILE]
                    src = msts[j][:]
                    if t == 0:
                        outs.append(nc.vector.tensor_copy(dst, src))
                    else:
                        outs.append(nc.vector.tensor_max(dst, dst, src))
                return outs

            # ---- node-phase lead-in (per 4-tile chunk): move odd-round half
            # down, fold max, relu+cbias into u_in; emitted as soon as the
            # chunk's aggr columns are final so it hides under the edge phase
            def emit_chunk(c):
                c0 = 4 * c * TILE
                cw = min(ncw - c0, 4 * TILE)
                d = nc.sync.dma_start(ah[:, c0:c0 + cw],
                                      aggr[64:128, c0:c0 + cw])
                tvc = nc.vector.tensor_copy(vscr[0:1, 1:2], bsb[0:1, 0:1])
                _dep(tvc, d, "DVE absorbs fold DMA dep")
                fo = nc.vector.tensor_max(ah[:, c0:c0 + cw],
                                          aggr[0:64, c0:c0 + cw],
                                          ah[:, c0:c0 + cw])
                _dep(fo, tvc, "order after absorber")
                ur = nc.scalar.activation(u_in[0:64, c0:c0 + cw],
                                          ah[:, c0:c0 + cw], AF.Relu,
                                          bias=w["cbias"], scale=1.0)
                return ur

            # =========== edge phase ===========
            zb_t = {}
            ms_t = {}
            rm_t = {}
            et_t = {}
            st_t = {}
            zb_t[0], ms_t[0] = emit_zbms(0, 0)
            zb_t[1], _ = emit_zbms(1, None)
            rm_t[0], et_t[0], prev_e = emit_re(0, zb_t[0], None)
            st_t[0] = emit_s(0, rm_t[0], et_t[0], None)
            prev_w2 = None

            chunks_done = set()
            chunk_insts = {}
            for g in range(G):
                # stage the superblock that groups g+2/g+3 will read
                c3 = (g + 3) * GRP
                new_sup = None
                if g + 3 < G and c3 % SUP == 0 and c3 // SUP < n_sup \
                        and sup_tiles[c3 // SUP] is None:
                    stage_sup(c3 // SUP)
                    new_sup = sup_dmas[c3 // SUP]
                # zb(g+2) || ms(g+1) concurrent pairs, pinned after the
                # previous iteration's w2 twin (which absorbed the sup DMA)
                gz = g + 2 if g + 2 < G else None
                gm = g + 1 if g + 1 < G else None
                if gz is not None or gm is not None:
                    zbt, msts = emit_zbms(gz, gm, after=prev_w2)
                    if gz is not None:
                        zb_t[gz] = zbt
                    if gm is not None:
                        ms_t[gm] = msts
                if g + 1 < G:
                    rm_t[g + 1], et_t[g + 1], prev_e = emit_re(
                        g + 1, zb_t[g + 1], prev_e)
                mm_e = emit_w2(g, ms_t.pop(g), st_t.pop(g))
                c3r = (g + 3) * GRP
                if g + 3 < G and c3r % SUP == 0 \
                        and sup_dmas[c3r // SUP] is not None:
                    _dep(mm_e[1], sup_dmas[c3r // SUP],
                         "sup prefetch via w2 twin")
                prev_w2 = mm_e[1]
                agg = emit_aggmax(g, None)
                if g + 1 < G:
                    st_t[g + 1] = emit_s(g + 1, rm_t[g + 1], et_t[g + 1],
                                         agg[0])
                rm_t.pop(g, None)
                et_t.pop(g, None)
                # early node-phase chunks once their blocks are final
                for c in range(1, n_ck):
                    if c not in chunks_done and ck_last[c] == g:
                        chunks_done.add(c)
                        chunk_insts[c] = emit_chunk(c)

            xdma = nc.sync.dma_start(u_in[64:67, :], xnode_d[:])
            chunk_insts[0] = emit_chunk(0)
            for c in range(1, n_ck):
                if c not in chunks_done:
                    chunk_insts[c] = emit_chunk(c)

            # absorber matmuls: observe the chunk-0 relu (ACT) and the xnode
            # DMA so the first node matmuls keep a single sem wait
            kwt1 = pz.tile([64, TILE], f32, tag="zb", name="kwt1")
            kw1 = nc.tensor.matmul(kwt1[:], wsb[0:64, 0:64],
                                   wsb[0:64, 0:TILE], start=True, stop=True)
            _dep(kw1, chunk_insts[0], "observe chunk-0 relu")
            kwt2 = pz.tile([64, TILE], f32, tag="zb", name="kwt2")
            kw2 = nc.tensor.matmul(kwt2[:], wsb[0:64, 0:64],
                                   wsb[0:64, 0:TILE], start=True, stop=True)
            _dep(kw2, xdma, "absorb xnode DMA wait")

            # =========== node phase ===========
            # per tile: zg = g1n@u; rg = relu(-zg-gb1); y2 = exp(zg+gb1);
            # sg = min(y2,1)+rg; o2 = g12@u + g2@sg; y = exp(o2+gbias);
            # vf2 = max(o2+gbias-1, -1); out = min(y,1)+vf2.
            # Explicit deps keep every instruction at <=1 sync wait.
            zg_t = {}
            o2_t = {}

            def emit_nmm1(i, prev_g2sg):
                ui = u_in[:, i * TILE:(i + 1) * TILE]
                zg = pz.tile([64, TILE], f32, tag="zb")
                nc.tensor.matmul(zg[:], w["g1n"], ui, start=True, stop=True)
                o2 = pm.tile([64, TILE], f32, tag="ms")
                o2i = nc.tensor.matmul(o2[:], w["g12"], ui, start=True,
                                       stop=False)
                if prev_g2sg is not None:
                    _dep(o2i, prev_g2sg, "pm slot DVE-WAR covered by g2sg")
                return zg, o2

            def emit_nact(i, prev_y2):
                rg = nrgpool.tile([64, TILE], bf16, tag="nrg")
                y2 = ny2pool.tile([64, TILE], bf16, tag="ny2")
                rgi = nc.scalar.activation(rg[:], zg_t[i][:], AF.Relu,
                                           bias=w["ngb1"], scale=-1.0)
                if prev_y2 is not None:
                    _dep(rgi, prev_y2, "nrg slot WAR covered by prev y2")
                y2i = nc.scalar.activation(y2[:], zg_t[i][:], AF.Exp,
                                           bias=w["pgb1"], scale=1.0)
                _dep(y2i, rgi, "share zg PE wait")
                return rg, y2, y2i

            def emit_nsg(i, rg, y2, prev_vf2):
                sg = nsgpool.tile([64, TILE], bf16, tag="nsg")
                sgi = nc.vector.scalar_tensor_tensor(
                    sg[:], y2[:], 1.0, rg[:], op0=ALU.min, op1=ALU.add)
                if prev_vf2 is not None:
                    _dep(sgi, prev_vf2, "nsg slot PE-WAR covered by vf2")
                return sg

            zg_t[0], o2_t[0] = emit_nmm1(0, None)
            rg_c, y2_c, y2i_c = emit_nact(0, None)
            sg_t = {0: emit_nsg(0, rg_c, y2_c, None)}
            prev_y2i = y2i_c
            prev_vf2 = None

            for i in range(n_nt):
                o2 = o2_t.pop(i)
                mm_o2 = nc.tensor.matmul(o2[:], w["g2"], sg_t.pop(i)[:],
                                         start=False, stop=True)
                if i + 1 < n_nt:
                    zg_t[i + 1], o2_t[i + 1] = emit_nmm1(i + 1, mm_o2)
                    rg_c, y2_c, y2i_c = emit_nact(i + 1, prev_y2i)
                    prev_y2i = y2i_c
                y = nypool.tile([64, TILE], bf16, tag="ny")
                yi = nc.scalar.activation(y[:], o2[:], AF.Exp,
                                          bias=w["gbias"], scale=1.0)
                _dep(yi, prev_y2i, "ny slot DVE-WAR covered by y2 wait")
                vf2 = nvfpool.tile([64, TILE], bf16, tag="nvf")
                vf2i = nc.vector.tensor_scalar(vf2[:], o2[:], w["gbm1"], -1.0,
                                               ALU.add, ALU.max)
                nc.vector.scalar_tensor_tensor(
                    out_sb[:, i * TILE:(i + 1) * TILE], y[:], 1.0, vf2[:],
                    op0=ALU.min, op1=ALU.add)
                if i + 1 < n_nt:
                    sg_t[i + 1] = emit_nsg(i + 1, rg_c, y2_c, vf2i)
                nc.sync.dma_start(out_d[:, i * TILE:(i + 1) * TILE],
                                  out_sb[:, i * TILE:(i + 1) * TILE])

    return nc


def _set_waits(inst, kept):
    """Replace an instruction's sync waits.  inst.sync_info returns a copy,
    so rebuild fresh SyncWait/SyncInfo objects and assign them back to the
    instruction."""
    import bass_rust
    news = [bass_rust.SyncWait(sync_type=x.sync_type, id=x.id,
                               wait_mode=x.wait_mode, ant_name=x.ant_name,
                               wait_value=x.wait_value, wait_reg=x.wait_reg)
            for x in kept]
    si = inst.sync_info
    ups = [bass_rust.SyncUpdate(sync_type=u.sync_type, id=u.id,
                                ant_name=u.ant_name, update_value=u.update_value)
           if False else u for u in list(si.on_update)]
    inst.sync_info = bass_rust.SyncInfo(on_wait=news, on_update=ups)


def _prune_waits(nc):
    """ISA structs carry at most one sync wait. Drop provably-redundant
    waits Tile emitted (same-engine self-waits on strict-FIFO engines;
    DMA-vs-DMA ordering subsumed by compute waits; drain-tail waits)."""
    n1 = n2 = 0
    for b in nc.m.functions[0].blocks:
        for i in b.instructions:
            si = i.sync_info
            if si is None or not si.on_wait or len(si.on_wait) < 2:
                continue
            nm = type(i).__name__
            waits = list(si.on_wait)
            if nm == "InstDrain":
                dma_w = [x for x in waits if x.ant_name.startswith("DMAHW")]
                _set_waits(i, dma_w[-1:] if dma_w else waits[-1:])
                continue
            if nm == "InstMatmult":
                act_w = [x for x in waits
                         if x.ant_name.startswith("Activation")]
                dma_w = [x for x in waits if x.ant_name.startswith("DMAHW")]
                if act_w and dma_w and len(act_w) + len(dma_w) == len(waits):
                    # sup-boundary zb matmul: its ACT slot-WAR (r read of the
                    # zb slot two groups back) is transitively enforced by the
                    # preceding w2 matmul's DVE wait (w2 <- s <- e <- r, ACT
                    # strict FIFO), so only the feats-DMA RAW must remain.
                    n1 += len(act_w)
                    _set_waits(i, dma_w)
                    continue
            if nm == "InstDMACopy":
                kept = [x for x in waits
                        if not (x.ant_name.startswith("DMAHW") or
                                x.ant_name.startswith("DMASW"))]
                if kept and len(kept) < len(waits):
                    n2 += len(waits) - len(kept)
                    _set_waits(i, kept)
                continue
            own = str(i.engine).split(".")[-1]
            kept = [x for x in waits
                    if x.ant_name.rsplit("_", 1)[0] != own]
            if len(kept) < len(waits):
                n1 += len(waits) - len(kept)
                _set_waits(i, kept)
    return n1, n2


# --------------------------------------------------------------------------
# entry points
# --------------------------------------------------------------------------

def _prepare(x, pos, edge_index, f_w1, f_b1, f_w2, f_b2,
             g_w1, g_b1, g_w2, g_b2):
    x = np.asarray(x, F32)
    pos = np.asarray(pos, F32)
    src = np.asarray(edge_index[0]).astype(np.int64)
    dst = np.asarray(edge_index[1]).astype(np.int64)
    cores = _core_layouts(edge_index)
    tiles, S, ncw = _tile_plan(cores)
    S_pad = ((S + SUP - 1) // SUP) * SUP
    packs = []
    for c, core in enumerate(cores):
        feats, xnode = _pack_core(core, tiles, S_pad, ncw, x, pos, src, dst)
        xnode[:, :NCN] = x[core["order"] + c * NCN].T
        packs.append((feats, xnode))
    w = _weights(np.asarray(f_w1, F32), np.asarray(f_b1, F32),
                 np.asarray(f_w2, F32), np.asarray(f_b2, F32),
                 np.asarray(g_w1, F32), np.asarray(g_b1, F32),
                 np.asarray(g_w2, F32), np.asarray(g_b2, F32))
    return cores, tiles, S_pad, ncw, packs, w


def _finalize(results, cores, x, g_w1, g_b1, g_w2, g_b2):
    """results: list of [64, ncw] per core -> full [N, 64] output."""
    out = np.empty((N, 64), dtype=F32)
    for c, core in enumerate(cores):
        out[core["order"] + c * NCN] = np.asarray(
            results[c], F32)[:, :NCN].T
    empties = np.concatenate([c["empty"] for c in cores])
    if empties.size:
        def celu(v):
            return np.maximum(v, 0) + np.minimum(0, np.expm1(np.minimum(v, 0)))
        u_in = np.concatenate(
            [np.zeros((empties.size, 64), F32), x[empties]], axis=1)
        u = celu(u_in @ g_w1 + g_b1)
        out[empties] = celu(u @ g_w2 + g_b2).astype(F32)
    return out


def kernel(x, pos, edge_index, f_w1, f_b1, f_w2, f_b2,
           g_w1, g_b1, g_w2, g_b2, _debug_numpy=False, _trace=False):
    x = np.asarray(x, F32)
    pos = np.asarray(pos, F32)
    cores, tiles, S_pad, ncw, packs, w = _prepare(
        x, pos, edge_index, f_w1, f_b1, f_w2, f_b2, g_w1, g_b1, g_w2, g_b2)

    if _debug_numpy:
        results = [_numpy_device(f, xn, w, tiles, ncw) for (f, xn) in packs]
        return _finalize(results, cores, x, np.asarray(g_w1, F32),
                         np.asarray(g_b1, F32), np.asarray(g_w2, F32),
                         np.asarray(g_b2, F32))

    _import_concourse()
    run_kwargs = {}
    if _trace:
        _install_ntff_shim()
        import concourse.bass_utils as _bu
        _bu.upload_artifacts = lambda tmpdir: f"file://{tmpdir}"
        import tempfile
        trace_dir = tempfile.mkdtemp(prefix="bass_trace_")
        run_kwargs = dict(tmpdir=trace_dir)
        kernel._last_trace_dir = trace_dir
    from concourse.bass_utils import run_bass_kernel_spmd

    import ml_dtypes
    bf = ml_dtypes.bfloat16
    nc = _build_nc(tiles, S_pad, ncw)
    _prune_waits(nc)
    in_maps = [{"feats": feats.astype(bf), "xnode": xnode.astype(bf),
                "wpack": w["wpack"], "bpack": w["bpack"]}
               for (feats, xnode) in packs]
    res = run_bass_kernel_spmd(nc, in_maps, list(range(CORES)), trace=_trace,
                               **run_kwargs)
    results = [res.results[c]["out"] for c in range(CORES)]
    out = _finalize(results, cores, x, np.asarray(g_w1, F32),
                    np.asarray(g_b1, F32), np.asarray(g_w2, F32),
                    np.asarray(g_b2, F32))
    if _trace:
        kernel._last_exec_time_ns = res.exec_time_ns
        kernel._last_mean_exec_time_ns = res.mean_exec_time_ns
    return out
